# revision 12
# baseline (speedup 1.0000x reference)
"""Multi-head attention (B=8, S=1024, D=1024, H=16) on 8 TRN2 NeuronCores.

Sharding: pure data-parallel over batch — core b computes batch b entirely
locally (no collectives). All matmuls run in bf16 with fp32 PSUM accumulation.

Per-core dataflow (host pre-transposes inputs/weights so no on-chip input
transposes are needed):
  Q_t[d,s], K_t[d,s] projected per d-tile (scale 1/sqrt(dk) folded into
  WQ/bq on the host; WQ/WK arrive as host-prearranged column slabs so they
  stream through SBUF instead of staying resident). V[s,d] is stored with a
  ones-column interleaved per head so the attention-value matmul also
  produces softmax row sums. Per head:
    S.T[k,q] = K_t_h.T @ Q_t_h   (single K=64 matmul per 512-chunk)
    E.T = exp(S.T) * mask.T      (ACT exp from PSUM; mask mult on GPSIMD)
    psum[q, 0:65] = sum_k E.T_tile.T @ [V_h | 1]  -> out + rowsum
    attnout[q, d_h] = psum[:,0:64] * recip(psum[:,64])   (DVE)
  attnout transposed via PE -> WO projection -> + bias -> out[s,o] fp32.

Schedule: V projection first (dense PE work while inputs stream in, psum
groups 4-way interleaved over the arriving wv/xv tiles). Main loop over
head pairs: each exp period (one [128,1024] ACT exp per score tile) gets
exactly 2 filler matmuls emitted inline — the next-next pair's Q-proj
fills head a's score phase, K-proj fills head b's (ProjJob), with AV of
the previous head pulled into the residue. Pairs 6,7 have no projections
left; they run WO pass 1 (contraction over aot[0..3]) as filler into bf16
SBUF accumulators reusing the idle xv pool, so the tail only runs WO pass
2 (aot[4..7]) + accumulator add, chunked 512-wide with psum alternating
between pools. PSUM: scores 2x2 banks, proj/WO 1x2, AV+transpose 2x1.
Masks multiply on GPSIMD/Pool (SBUF only — Pool cannot touch PSUM).

Cost model: ~213us single / ~200us/iter steady-state at 92% PE occupancy;
PE busy 196.5us is the column floor (every matmul group sits at the PE
output-bandwidth bound of 128 out-elems per column x contraction tiles).
Real HW runs PE 1.29x slower than the model -> ~253us/iter floor,
measured ~257us marginal (chained-iters method, see test.py).
"""

import os
import sys
from contextlib import ExitStack

import numpy as np

if os.environ.get("JAX_PLATFORMS") == "cpu":
    # bass execution needs the neuron/axon jax backend
    del os.environ["JAX_PLATFORMS"]

for _p in ("/opt/trn_rl_repo",):
    if _p not in sys.path and os.path.isdir(_p):
        sys.path.insert(0, _p)

import ml_dtypes

import concourse.bass as bass
import concourse.mybir as mybir
import concourse.tile as tile
from concourse import bacc
from concourse.bass import ds, ts
from concourse.bass_utils import run_bass_kernel_spmd
from concourse.masks import make_identity

BF16 = mybir.dt.bfloat16
F32 = mybir.dt.float32
NPBF = ml_dtypes.bfloat16

B, S, D, H, DK = 8, 1024, 1024, 16, 64
P = 128
NT = D // P  # 8 tiles along any 1024 dim
CH = 512  # matmul moving-dim chunk (one PSUM bank of fp32)
NCH = S // CH  # 2

MASK_ON_GPSIMD = True

LAST_RESULTS = None
_NC_CACHE = None


def build_nc(iters=1, mask_on_gpsimd=MASK_ON_GPSIMD):
    nc = bacc.Bacc("TRN2", target_bir_lowering=False, debug=False)

    xq = nc.dram_tensor("xq", [D, S], BF16, kind="ExternalInput")  # q[b].T
    xk = nc.dram_tensor("xk", [D, S], BF16, kind="ExternalInput")
    xv = nc.dram_tensor("xv", [D, S], BF16, kind="ExternalInput")
    # wq/wk: host-prearranged column slabs [t][p][i*128+f] = W.T[i*128+p, t*128+f]
    wq = nc.dram_tensor("wq", [NT, P, D], BF16, kind="ExternalInput")
    wk = nc.dram_tensor("wk", [NT, P, D], BF16, kind="ExternalInput")
    wv = nc.dram_tensor("wv", [D, D], BF16, kind="ExternalInput")  # WV_w.T
    wo = nc.dram_tensor("wo", [D, D], BF16, kind="ExternalInput")  # WO_w.T
    bq = nc.dram_tensor("bq", [P, NT], F32, kind="ExternalInput")  # WQ_b/8
    bk = nc.dram_tensor("bk", [P, NT], F32, kind="ExternalInput")
    bvb = nc.dram_tensor("bvb", [P, H * 65], BF16, kind="ExternalInput")
    bob = nc.dram_tensor("bob", [P, D], BF16, kind="ExternalInput")
    mt = nc.dram_tensor("mt", [S, S], BF16, kind="ExternalInput")  # mask[b,0].T
    out = nc.dram_tensor("out", [S, D], BF16, kind="ExternalOutput")

    with tile.TileContext(nc) as tc, ExitStack() as ctx:
        pers = ctx.enter_context(tc.tile_pool(name="pers", bufs=1))
        # xq+xk resident for the whole kernel
        xld = ctx.enter_context(tc.tile_pool(name="xld", bufs=16))
        xvp = ctx.enter_context(tc.tile_pool(name="xvp", bufs=8))
        # wv and wo each get dedicated slots: sharing one pool couples the
        # next iteration's wv DMAs to this iteration's WO tail (SP issues
        # descriptors in order, so one waiting DMA head-of-line blocks all
        # later input loads at the iteration boundary)
        wld = ctx.enter_context(tc.tile_pool(name="wld", bufs=8))
        wold = ctx.enter_context(tc.tile_pool(name="wold", bufs=8))
        wslab = ctx.enter_context(tc.tile_pool(name="wslab", bufs=3))
        # WO pass-1 accumulators get their OWN pool: when they borrowed xvp
        # slots, the next iteration's xv DMAs blocked on this iteration's
        # TAIL, and the in-order SP queue then stalled the whole next input
        # stream (no cross-iteration prefetch).
        wacc = ctx.enter_context(tc.tile_pool(name="wacc", bufs=8))
        # q/k projection outputs: only live for their head pair -> rotate
        qkp = ctx.enter_context(tc.tile_pool(name="qkp", bufs=3))
        epool = ctx.enter_context(tc.tile_pool(name="epool", bufs=16))
        aop = ctx.enter_context(tc.tile_pool(name="aop", bufs=16))
        opool = ctx.enter_context(tc.tile_pool(name="opool", bufs=2))
        rpool = ctx.enter_context(tc.tile_pool(name="rpool", bufs=8))
        # psum: scores get their own 2-slot pool (2 banks each) so the
        # in-loop projection (1-slot ppj pool) never steals a slot the
        # exp pipeline is about to recycle; ps_av keeps 2 one-bank slots
        # for AV accumulation + attnout transposes. 2*2 + 1*2 + 2*1 = 8.
        pst = ctx.enter_context(tc.tile_pool(name="pst", bufs=2, space="PSUM"))
        ppj = ctx.enter_context(tc.tile_pool(name="ppj", bufs=1, space="PSUM"))
        ps_av = ctx.enter_context(tc.tile_pool(name="ps_av", bufs=2, space="PSUM"))

        # ---- persistent tiles ----
        vv = [
            pers.tile([P, H * 65], BF16, name=f"vv{t}", tag=f"vv{t}")
            for t in range(NT)
        ]
        msk = [pers.tile([P, S], BF16, name=f"mk{t}", tag=f"mk{t}") for t in range(NT)]
        aot = [pers.tile([P, S], BF16, name=f"at{t}", tag=f"at{t}") for t in range(NT)]
        ident = pers.tile([P, P], BF16, name="ident", tag="ident")
        bq_sb = pers.tile([P, NT], F32, name="bq_sb", tag="bq_sb")
        bk_sb = pers.tile([P, NT], F32, name="bk_sb", tag="bk_sb")
        bv_sb = pers.tile([P, H * 65], BF16, name="bv_sb", tag="bv_sb")
        bo_sb = pers.tile([P, D], BF16, name="bo_sb", tag="bo_sb")

        make_identity(nc, ident)

        def emit_body():
            nc.sync.dma_start(bq_sb[:], bq[:])
            nc.sync.dma_start(bk_sb[:], bk[:])

            def load_slab(wdram, ot):
                wsl = wslab.tile([P, D], BF16, name="wsl", tag="ws")
                nc.sync.dma_start(wsl[:], wdram[ot])
                return wsl

            # ---- input DMAs: V-path first — V-proj is ~2x denser PE work per DMA
            # byte than the Q/K path, so it best fills the DMA-paced startup.
            # x/slab/mask DMAs land while V-proj computes. ----
            wvsb = []
            xvsb = []
            for i in range(NT):
                w_t = wld.tile([P, D], BF16, name=f"wv{i}", tag="w")
                nc.sync.dma_start(w_t[:], wv[ts(i, P), :])
                wvsb.append(w_t)
                x_t = xvp.tile([P, S], BF16, name=f"xv{i}", tag="xv")
                nc.sync.dma_start(x_t[:], xv[ts(i, P), :])
                xvsb.append(x_t)
                if i == 0:
                    nc.sync.dma_start(bv_sb[:], bvb[:])
            sl_q = load_slab(wq, 0)
            sl_k = load_slab(wk, 0)
            xqsb, xksb = [], []
            for i in range(NT):
                x_t = xld.tile([P, S], BF16, name=f"xq{i}", tag="x")
                nc.sync.dma_start(x_t[:], xq[ts(i, P), :])
                xqsb.append(x_t)
                x_t = xld.tile([P, S], BF16, name=f"xk{i}", tag="x")
                nc.sync.dma_start(x_t[:], xk[ts(i, P), :])
                xksb.append(x_t)
            for i in range(NT):
                nc.sync.dma_start(msk[i][:], mt[ts(i, P), :])
            nc.sync.dma_start(bo_sb[:], bob[:])

            def project(wsl, bias, ot, xtiles, pname, pool=None):
                ps = (pool or pst).tile([P, S], F32, name="ps_pj", tag="st")
                # i outer / c inner: consecutive matmuls share the stationary
                # operand, so the redundant LDWEIGHTS is elided (HW ~130ns/MM
                # vs ~190 with a fresh stationary every MM)
                for i in range(NT):
                    for c in range(NCH):
                        nc.tensor.matmul(
                            ps[:, ts(c, CH)],
                            wsl[:, ts(i, P)],
                            xtiles[i][:, ts(c, CH)],
                            start=(i == 0),
                            stop=(i == NT - 1),
                        )
                dst = qkp.tile([P, S], BF16, name=pname, tag=pname[0])
                # two chunk evictions so the next pair's first QK matmuls (which
                # read chunk 0) unblock ~0.5us earlier
                for c in range(NCH):
                    nc.vector.tensor_scalar_add(
                        dst[:, ts(c, CH)], ps[:, ts(c, CH)], bias[:, ds(ot, 1)]
                    )
                return dst

            class ProjJob:
                """A projection whose 16 matmuls are doled out as PE filler
                between score tiles (each ~213ns matmul fills the gap the
                1038ns exp leaves per score period). Uses the 1-slot ppj
                psum pool so it never contends with the score tiles."""

                def __init__(self, wsl, bias, ot, xtiles, pname):
                    self.ps = ppj.tile([P, S], F32, name="ps_pj", tag="pj")
                    self.wsl, self.bias, self.ot = wsl, bias, ot
                    self.xtiles, self.pname = xtiles, pname
                    # i outer / c inner: stationary reuse between the two
                    # chunks of each i (LDWEIGHTS elision)
                    self.steps = [(c, i) for i in range(NT) for c in range(NCH)]
                    self.dst = None

                def step(self):
                    c, i = self.steps.pop(0)
                    nc.tensor.matmul(
                        self.ps[:, ts(c, CH)],
                        self.wsl[:, ts(i, P)],
                        self.xtiles[i][:, ts(c, CH)],
                        start=(i == 0),
                        stop=(i == NT - 1),
                    )
                    if i == NT - 1:
                        # per-chunk eviction: frees the 1-slot ppj pool for
                        # the next job sooner (slot handoff waits on all
                        # evictions of the previous tenant)
                        if self.dst is None:
                            self.dst = qkp.tile(
                                [P, S], BF16, name=self.pname, tag=self.pname[0]
                            )
                        nc.vector.tensor_scalar_add(
                            self.dst[:, ts(c, CH)],
                            self.ps[:, ts(c, CH)],
                            self.bias[:, ds(self.ot, 1)],
                        )
                    return bool(self.steps)

            class WoPass1Job:
                """First half of WO s-tile j: ps = sum_{i<4} aot[i].T-slice @
                WO-slab, evicted (with bias) to a bf16 SBUF accumulator in the
                long-idle xvp pool. Doled out as PE filler during pairs 6,7
                (which have no projection work left); the tail then only runs
                i=4..7 + the accumulator add."""

                def __init__(self, j, wosb, woacc):
                    self.ps = ppj.tile([P, D], F32, name="ps_w1", tag="pj")
                    self.j, self.wosb, self.woacc = j, wosb, woacc
                    self.steps = [(c, i) for i in range(4) for c in range(NCH)]
                    self.dst = None

                def step(self):
                    c, i = self.steps.pop(0)
                    nc.tensor.matmul(
                        self.ps[:, ts(c, CH)],
                        aot[i][:, ts(self.j, P)],
                        self.wosb[i][:, ts(c, CH)],
                        start=(i == 0),
                        stop=(i == 3),
                    )
                    if not self.steps:
                        acc = wacc.tile([P, D], BF16, name=f"wa{self.j}", tag="wa")
                        nc.vector.tensor_add(acc[:], self.ps[:], bo_sb[:])
                        self.woacc[self.j] = acc
                    return bool(self.steps)

            def head_qk(h, qt_t, kt_t, jobs):
                """scores -> exp -> mask for head h; returns the 8 E.T tiles.
                After each score tile, emits 2 filler matmuls from `jobs`."""
                prow = (h % 2) * 64
                eh = []
                for i in range(NT):
                    st_ps = pst.tile([P, S], F32, name="st", tag="st")
                    for c in range(NCH):
                        nc.tensor.matmul(
                            st_ps[:, ts(c, CH)],
                            kt_t[ds(prow, 64), ts(i, P)],
                            qt_t[ds(prow, 64), ts(c, CH)],
                            start=True,
                            stop=True,
                        )
                    e = epool.tile([P, S], BF16, name=f"e{i}", tag="e")
                    nc.scalar.activation(e[:], st_ps[:], mybir.ActivationFunctionType.Exp)
                    # masks all on GPSIMD: keeps the DVE queue free of ACT-paced
                    # work so the q/k projection evictions (which gate the next
                    # pair's QK) run as soon as their psum completes
                    if mask_on_gpsimd:
                        nc.gpsimd.tensor_mul(e[:], e[:], msk[i][:])
                    else:
                        nc.vector.tensor_mul(e[:], e[:], msk[i][:])
                    eh.append(e)
                    for _ in range(2):
                        if jobs and not jobs[0].step():
                            jobs.pop(0)
                return eh

            def head_av(h, eh, aopair):
                prow = (h % 2) * 64
                for j in range(NT):
                    av = ps_av.tile([P, P], F32, name="av", tag="av")
                    for i in range(NT):
                        nc.tensor.matmul(
                            av[:, 0:65],
                            eh[i][:, ts(j, P)],
                            vv[i][:, ds(h * 65, 65)],
                            start=(i == 0),
                            stop=(i == NT - 1),
                        )
                    rc = rpool.tile([P, 1], F32, name="rc", tag="rc")
                    nc.vector.reciprocal(rc[:], av[:, ds(64, 1)])
                    nc.vector.tensor_scalar_mul(
                        aopair[j][:, ds(prow, 64)], av[:, 0:64], rc[:]
                    )

            def transpose_pair(t, aopair):
                # all 8 [128,128]bf16 transposes fit ONE psum bank: 1 slot + 1 big
                # DVE copy instead of 8 of each. (DMA-xbar transposes measured
                # WORSE: they contend with the chained input prefetch on the
                # in-order SP queue.)
                ptb = ps_av.tile([P, S], BF16, name="ptb", tag="av")
                for j in range(NT):
                    nc.tensor.matmul(
                        ptb[:, ts(j, P)],
                        aopair[j][:],
                        ident[:],
                        is_transpose=True,
                        start=(j == 0),
                        stop=(j == NT - 1),
                        skip_group_check=True,
                    )
                nc.vector.tensor_copy(aot[t][:], ptb[:])

            # ---- V projection first (dense PE work during input DMA; AV depends
            # on all of V) ----
            # V[s, d]: stationary = x.T [i,s]-tile, moving = W.T [i,o].
            # Groups are processed 3 at a time (= pp bufs) with their i-loops
            # interleaved: the stream is paced by each wv/xv tile's DMA arrival,
            # so 3-way interleave gives PE 3x the work per arriving tile.
            # 4 groups per batch: 3 through pp (2-bank tiles) + 1 split into its
            # two 512-chunks through the ps_av slots (1 bank each, idle during
            # V-proj) -> 8 matmuls per arriving wv/xv tile instead of 6 during
            # the DMA-paced phase, and 2 batches instead of 3.
            for g0 in (0, 4):
                batch = list(range(g0, g0 + 4))
                pss = {}
                for st_ in batch:
                    nc.gpsimd.memset(
                        vv[st_].rearrange("p (g c) -> p g c", c=65)[:, :, 64:65], 1.0
                    )
                pss[batch[0]] = pst.tile([P, D], F32, name="ps_pv", tag="st")
                pss[batch[1]] = pst.tile([P, D], F32, name="ps_pv", tag="st")
                pss[batch[2]] = ppj.tile([P, D], F32, name="ps_pv", tag="pj")
                sp = batch[3]
                half = [
                    ps_av.tile([P, CH], F32, name="ps_ph", tag="av") for _ in range(NCH)
                ]
                for i in range(NT):
                    for st_ in batch[:3]:
                        for c in range(NCH):
                            nc.tensor.matmul(
                                pss[st_][:, ts(c, CH)],
                                xvsb[i][:, ts(st_, P)],
                                wvsb[i][:, ts(c, CH)],
                                start=(i == 0),
                                stop=(i == NT - 1),
                            )
                    for c in range(NCH):
                        nc.tensor.matmul(
                            half[c][:],
                            xvsb[i][:, ts(sp, P)],
                            wvsb[i][:, ts(c, CH)],
                            start=(i == 0),
                            stop=(i == NT - 1),
                        )
                for st_ in batch[:3]:
                    # scatter 16 head-blocks of 64 into 65-strided layout, + bias
                    nc.vector.tensor_add(
                        vv[st_].rearrange("p (g c) -> p g c", c=65)[:, :, 0:64],
                        pss[st_].rearrange("p (g c) -> p g c", c=64),
                        bv_sb.rearrange("p (g c) -> p g c", c=65)[:, :, 0:64],
                    )
                for c in range(NCH):
                    g0c = c * 8
                    nc.vector.tensor_add(
                        vv[sp][:, ds(g0c * 65, 8 * 65)].rearrange(
                            "p (g c) -> p g c", c=65
                        )[:, :, 0:64],
                        half[c].rearrange("p (g c) -> p g c", c=64),
                        bv_sb[:, ds(g0c * 65, 8 * 65)].rearrange(
                            "p (g c) -> p g c", c=65
                        )[:, :, 0:64],
                    )

            # ---- main loop over head pairs ----
            # static PE order per pair: QK (feeds ACT) -> next-pair projection
            # (fills PE while ACT runs the exps) -> AV(2t) -> previous pair's
            # transposes (extra PE filler before AV(2t+1)'s E is ready) -> AV(2t+1)
            def wo_stile2(j, wosb, woacc):
                # second half (i=4..7) of WO s-tile j + the pass-1 accumulator.
                # psum alternates pst/ppj so eviction of tile j overlaps the
                # matmuls of tile j+1 (scores/projections are done by now).
                pool, tag = (pst, "st") if j % 2 else (ppj, "pj")
                ps = pool.tile([P, D], F32, name="ps_wo", tag=tag)
                for i in range(4, NT):
                    for c in range(NCH):
                        nc.tensor.matmul(
                            ps[:, ts(c, CH)],
                            aot[i][:, ts(j, P)],
                            wosb[i][:, ts(c, CH)],
                            start=(i == 4),
                            stop=(i == NT - 1),
                        )
                # final add + out DMA in 512-chunks: halves the last tile's
                # serial evict->DMA latency (adds on DVE — Pool can't read PSUM)
                osb = opool.tile([P, D], BF16, name="osb", tag="osb")
                for c in range(NCH):
                    nc.vector.tensor_add(
                        osb[:, ts(c, CH)], ps[:, ts(c, CH)], woacc[j][:, ts(c, CH)]
                    )
                    nc.sync.dma_start(out[ts(j, P), ts(c, CH)], osb[:, ts(c, CH)])

            # projections run two pairs ahead (qkp bufs=3 per tag: current, next,
            # next-next) so iteration 0's AVs have a full exp-latency of PE filler
            qts = {0: project(sl_q, bq_sb, 0, xqsb, "qt")}
            kts = {0: project(sl_k, bk_sb, 0, xksb, "kt")}
            qts[1] = project(load_slab(wq, 1), bq_sb, 1, xqsb, "qt")
            kts[1] = project(load_slab(wk, 1), bk_sb, 1, xksb, "kt")

            prev = None
            wosb = []
            woacc = [None] * NT
            for t in range(NT):
                qt_t, kt_t = qts.pop(t), kts.pop(t)
                aopair = [
                    aop.tile([P, P], BF16, name=f"aop{j}", tag="aop") for j in range(NT)
                ]
                # next-next pair's projections ride along as per-period PE
                # filler inside the score phases (2 matmuls per exp period):
                # q-proj fills head a's phase, k-proj fills head b's. Pairs
                # 6,7 have no projections left; they get WO pass-1 instead
                # (4 j-tiles per pair x 8 steps = the same 32 slots).
                jobs = []
                jq = jk = None
                if t + 2 < NT:
                    jq = ProjJob(load_slab(wq, t + 2), bq_sb, t + 2, xqsb, "qt")
                    jobs.append(jq)
                else:
                    jobs.extend(
                        WoPass1Job(j, wosb, woacc)
                        for j in range(4 * (t - (NT - 2)), 4 * (t - (NT - 2)) + 2)
                    )
                eh_a = head_qk(2 * t, qt_t, kt_t, jobs)
                if t + 2 < NT:
                    jk = ProjJob(load_slab(wk, t + 2), bk_sb, t + 2, xksb, "kt")
                    jobs.append(jk)
                else:
                    jobs.extend(
                        WoPass1Job(j, wosb, woacc)
                        for j in range(4 * (t - (NT - 2)) + 2, 4 * (t - (NT - 2)) + 4)
                    )
                eh_b = head_qk(2 * t + 1, qt_t, kt_t, jobs)
                while jobs:  # safety drain (steps == slots normally)
                    if not jobs[0].step():
                        jobs.pop(0)
                if t + 2 < NT:
                    qts[t + 2] = jq.dst
                    kts[t + 2] = jk.dst
                if t == 4:
                    # prefetch WO weights
                    for i in range(NT):
                        w_t = wold.tile([P, D], BF16, name=f"wo{i}", tag="wo")
                        nc.sync.dma_start(w_t[:], wo[ts(i, P), :])
                        wosb.append(w_t)
                if prev is not None:
                    transpose_pair(t - 1, prev)
                head_av(2 * t, eh_a, aopair)
                if t < NT - 1:
                    head_av(2 * t + 1, eh_b, aopair)
                else:
                    # tail: interleave the last head's AV with its transposes and
                    # the WO s-tiles so the output projection starts per-j instead
                    # of waiting for the whole pair
                    prow = 64
                    for j in range(NT + 2):
                        if j < NT:
                            av = ps_av.tile([P, P], F32, name="av", tag="av")
                            for i in range(NT):
                                nc.tensor.matmul(
                                    av[:, 0:65],
                                    eh_b[i][:, ts(j, P)],
                                    vv[i][:, ds((2 * t + 1) * 65, 65)],
                                    start=(i == 0),
                                    stop=(i == NT - 1),
                                )
                            rc = rpool.tile([P, 1], F32, name="rc", tag="rc")
                            nc.vector.reciprocal(rc[:], av[:, ds(64, 1)])
                            nc.vector.tensor_scalar_mul(
                                aopair[j][:, ds(prow, 64)], av[:, 0:64], rc[:]
                            )
                        if 1 <= j <= NT:
                            pt = ps_av.tile([P, P], BF16, name="pt", tag="av")
                            nc.tensor.transpose(pt[:], aopair[j - 1][:], ident[:])
                            nc.vector.tensor_copy(aot[t][:, ts(j - 1, P)], pt[:])
                        if j >= 2:
                            wo_stile2(j - 2, wosb, woacc)
                prev = aopair

        for _it in range(iters):
            emit_body()

    nc.compile()
    return nc


def prep_inputs(q, k, v, mask, WQ_w, WQ_b, WK_w, WK_b, WV_w, WV_b, WO_w, WO_b):
    """Build the 8 per-core input maps (host-side layout prep)."""
    f32 = np.float32

    def slabs(wt):  # [D,D] W.T -> [NT, P, D]: [t][p][i*128+f] = wt[i*128+p, t*128+f]
        return np.ascontiguousarray(
            wt.reshape(NT, P, NT, P).transpose(2, 1, 0, 3).reshape(NT, P, D)
        )

    wq_t = slabs((WQ_w.astype(f32) * 0.125).T).astype(NPBF)
    wk_t = slabs(WK_w.astype(f32).T).astype(NPBF)
    wv_t = np.ascontiguousarray(WV_w.astype(f32).T).astype(NPBF)
    wo_t = np.ascontiguousarray(WO_w.astype(f32).T).astype(NPBF)
    bq_l = np.ascontiguousarray((WQ_b.astype(f32) * 0.125).reshape(NT, P).T)
    bk_l = np.ascontiguousarray(WK_b.astype(f32).reshape(NT, P).T)
    bvb = np.zeros((P, H * 65), NPBF)
    bv_f = WV_b.astype(f32)
    for h in range(H):
        bvb[:, h * 65 : h * 65 + 64] = bv_f[h * 64 : (h + 1) * 64].astype(NPBF)[None, :]
    bob = np.ascontiguousarray(np.broadcast_to(WO_b.astype(f32), (P, D))).astype(NPBF)

    in_maps = []
    for b in range(B):
        in_maps.append(
            {
                "xq": np.ascontiguousarray(q[b].astype(f32).T).astype(NPBF),
                "xk": np.ascontiguousarray(k[b].astype(f32).T).astype(NPBF),
                "xv": np.ascontiguousarray(v[b].astype(f32).T).astype(NPBF),
                "wq": wq_t,
                "wk": wk_t,
                "wv": wv_t,
                "wo": wo_t,
                "bq": bq_l,
                "bk": bk_l,
                "bvb": bvb,
                "bob": bob,
                "mt": np.ascontiguousarray(mask[b, 0].T.astype(f32)).astype(NPBF),
            }
        )
    return in_maps


def _ensure_neuron_backend():
    # if jax was already initialized cpu-only (e.g. JAX_PLATFORMS=cpu was set
    # before this module was imported), re-discover the neuron/axon backend
    import jax

    try:
        if all(d.platform == "cpu" for d in jax.devices()):
            jax.clear_backends()
    except Exception:
        pass


def kernel(q, k, v, mask, WQ_w, WQ_b, WK_w, WK_b, WV_w, WV_b, WO_w, WO_b):
    global _NC_CACHE, LAST_RESULTS
    _ensure_neuron_backend()
    if _NC_CACHE is None:
        _NC_CACHE = build_nc()
    nc = _NC_CACHE
    in_maps = prep_inputs(
        q, k, v, mask, WQ_w, WQ_b, WK_w, WK_b, WV_w, WV_b, WO_w, WO_b
    )
    res = run_bass_kernel_spmd(nc, in_maps, core_ids=list(range(B)))
    LAST_RESULTS = res
    return np.stack([res.results[b]["out"] for b in range(B)], axis=0).astype(
        np.float32
    )



# revision 13
# speedup vs baseline: 1.0110x; 1.0110x over previous
"""Multi-head attention (B=8, S=1024, D=1024, H=16) on 8 TRN2 NeuronCores.

Sharding: pure data-parallel over batch — core b computes batch b entirely
locally (no collectives). All matmuls run in bf16 with fp32 PSUM accumulation.

Per-core dataflow (host pre-transposes inputs/weights so no on-chip input
transposes are needed):
  Q_t[d,s], K_t[d,s] projected per d-tile (scale 1/sqrt(dk) folded into
  WQ/bq on the host; WQ/WK arrive as host-prearranged column slabs so they
  stream through SBUF instead of staying resident). V[s,d] is stored with a
  ones-column interleaved per head so the attention-value matmul also
  produces softmax row sums. Per head:
    S.T[k,q] = K_t_h.T @ Q_t_h   (single K=64 matmul per 512-chunk)
    E.T = exp(S.T) * mask.T      (ACT exp from PSUM; mask mult on GPSIMD)
    psum[q, 0:65] = sum_k E.T_tile.T @ [V_h | 1]  -> out + rowsum
    attnout[q, d_h] = psum[:,0:64] * recip(psum[:,64])   (DVE)
  attnout transposed via PE -> WO projection -> + bias -> out[s,o] fp32.

Schedule: V projection first (dense PE work while inputs stream in, psum
groups 4-way interleaved over the arriving wv/xv tiles). Main loop over
head pairs: each exp period (one [128,1024] ACT exp per score tile) gets
exactly 2 filler matmuls emitted inline — the next-next pair's Q-proj
fills head a's score phase, K-proj fills head b's (ProjJob), with AV of
the previous head pulled into the residue. Pairs 6,7 have no projections
left; they run WO pass 1 (contraction over aot[0..3]) as filler into bf16
SBUF accumulators reusing the idle xv pool, so the tail only runs WO pass
2 (aot[4..7]) + accumulator add, chunked 512-wide with psum alternating
between pools. PSUM: scores 2x2 banks, proj/WO 1x2, AV+transpose 2x1.
Masks multiply on GPSIMD/Pool (SBUF only — Pool cannot touch PSUM).

Cost model: ~213us single / ~200us/iter steady-state at 92% PE occupancy;
PE busy 196.5us is the column floor (every matmul group sits at the PE
output-bandwidth bound of 128 out-elems per column x contraction tiles).
Real HW runs PE 1.29x slower than the model -> ~253us/iter floor,
measured ~257us marginal (chained-iters method, see test.py).
"""

import os
import sys
from contextlib import ExitStack

import numpy as np

if os.environ.get("JAX_PLATFORMS") == "cpu":
    # bass execution needs the neuron/axon jax backend
    del os.environ["JAX_PLATFORMS"]

for _p in ("/opt/trn_rl_repo",):
    if _p not in sys.path and os.path.isdir(_p):
        sys.path.insert(0, _p)

import ml_dtypes

import concourse.bass as bass
import concourse.mybir as mybir
import concourse.tile as tile
from concourse import bacc
from concourse.bass import ds, ts
from concourse.bass_utils import run_bass_kernel_spmd
from concourse.masks import make_identity

BF16 = mybir.dt.bfloat16
F32 = mybir.dt.float32
NPBF = ml_dtypes.bfloat16

B, S, D, H, DK = 8, 1024, 1024, 16, 64
P = 128
NT = D // P  # 8 tiles along any 1024 dim
CH = 512  # matmul moving-dim chunk (one PSUM bank of fp32)
NCH = S // CH  # 2

MASK_ON_GPSIMD = True

LAST_RESULTS = None
_NC_CACHE = None


def build_nc(iters=1, mask_on_gpsimd=MASK_ON_GPSIMD):
    nc = bacc.Bacc("TRN2", target_bir_lowering=False, debug=False)

    xq = nc.dram_tensor("xq", [D, S], BF16, kind="ExternalInput")  # q[b].T
    xk = nc.dram_tensor("xk", [D, S], BF16, kind="ExternalInput")
    xv = nc.dram_tensor("xv", [D, S], BF16, kind="ExternalInput")
    # wq/wk: host-prearranged column slabs [t][p][i*128+f] = W.T[i*128+p, t*128+f]
    wq = nc.dram_tensor("wq", [NT, P, D], BF16, kind="ExternalInput")
    wk = nc.dram_tensor("wk", [NT, P, D], BF16, kind="ExternalInput")
    wv = nc.dram_tensor("wv", [D, D], BF16, kind="ExternalInput")  # WV_w.T
    wo = nc.dram_tensor("wo", [D, D], BF16, kind="ExternalInput")  # WO_w.T
    bq = nc.dram_tensor("bq", [P, NT], F32, kind="ExternalInput")  # WQ_b/8
    bk = nc.dram_tensor("bk", [P, NT], F32, kind="ExternalInput")
    bvb = nc.dram_tensor("bvb", [P, H * 65], BF16, kind="ExternalInput")
    bob = nc.dram_tensor("bob", [P, D], F32, kind="ExternalInput")
    mt = nc.dram_tensor("mt", [S, S], BF16, kind="ExternalInput")  # mask[b,0].T
    out = nc.dram_tensor("out", [S, D], F32, kind="ExternalOutput")

    with tile.TileContext(nc) as tc, ExitStack() as ctx:
        pers = ctx.enter_context(tc.tile_pool(name="pers", bufs=1))
        # xq+xk resident for the whole kernel
        xld = ctx.enter_context(tc.tile_pool(name="xld", bufs=16))
        xvp = ctx.enter_context(tc.tile_pool(name="xvp", bufs=8))
        # wv and wo each get dedicated slots: sharing one pool couples the
        # next iteration's wv DMAs to this iteration's WO tail (SP issues
        # descriptors in order, so one waiting DMA head-of-line blocks all
        # later input loads at the iteration boundary)
        wld = ctx.enter_context(tc.tile_pool(name="wld", bufs=8))
        wold = ctx.enter_context(tc.tile_pool(name="wold", bufs=8))
        wslab = ctx.enter_context(tc.tile_pool(name="wslab", bufs=4))
        # q/k projection outputs: only live for their head pair -> rotate
        qkp = ctx.enter_context(tc.tile_pool(name="qkp", bufs=3))
        epool = ctx.enter_context(tc.tile_pool(name="epool", bufs=16))
        aop = ctx.enter_context(tc.tile_pool(name="aop", bufs=16))
        opool = ctx.enter_context(tc.tile_pool(name="opool", bufs=2))
        rpool = ctx.enter_context(tc.tile_pool(name="rpool", bufs=8))
        # psum: scores get their own 2-slot pool (2 banks each) so the
        # in-loop projection (1-slot ppj pool) never steals a slot the
        # exp pipeline is about to recycle; ps_av keeps 2 one-bank slots
        # for AV accumulation + attnout transposes. 2*2 + 1*2 + 2*1 = 8.
        pst = ctx.enter_context(tc.tile_pool(name="pst", bufs=2, space="PSUM"))
        ppj = ctx.enter_context(tc.tile_pool(name="ppj", bufs=1, space="PSUM"))
        ps_av = ctx.enter_context(tc.tile_pool(name="ps_av", bufs=2, space="PSUM"))

        # ---- persistent tiles ----
        vv = [
            pers.tile([P, H * 65], BF16, name=f"vv{t}", tag=f"vv{t}")
            for t in range(NT)
        ]
        msk = [pers.tile([P, S], BF16, name=f"mk{t}", tag=f"mk{t}") for t in range(NT)]
        aot = [pers.tile([P, S], BF16, name=f"at{t}", tag=f"at{t}") for t in range(NT)]
        ident = pers.tile([P, P], BF16, name="ident", tag="ident")
        bq_sb = pers.tile([P, NT], F32, name="bq_sb", tag="bq_sb")
        bk_sb = pers.tile([P, NT], F32, name="bk_sb", tag="bk_sb")
        bv_sb = pers.tile([P, H * 65], BF16, name="bv_sb", tag="bv_sb")
        bo_sb = pers.tile([P, D], F32, name="bo_sb", tag="bo_sb")

        make_identity(nc, ident)

        def emit_body():
            nc.sync.dma_start(bq_sb[:], bq[:])
            nc.sync.dma_start(bk_sb[:], bk[:])

            def load_slab(wdram, ot):
                wsl = wslab.tile([P, D], BF16, name="wsl", tag="ws")
                nc.sync.dma_start(wsl[:], wdram[ot])
                return wsl

            # ---- input DMAs: V-path first — V-proj is ~2x denser PE work per DMA
            # byte than the Q/K path, so it best fills the DMA-paced startup.
            # x/slab/mask DMAs land while V-proj computes. ----
            wvsb = []
            xvsb = []
            for i in range(NT):
                w_t = wld.tile([P, D], BF16, name=f"wv{i}", tag="w")
                nc.sync.dma_start(w_t[:], wv[ts(i, P), :])
                wvsb.append(w_t)
                x_t = xvp.tile([P, S], BF16, name=f"xv{i}", tag="xv")
                nc.sync.dma_start(x_t[:], xv[ts(i, P), :])
                xvsb.append(x_t)
                if i == 0:
                    nc.sync.dma_start(bv_sb[:], bvb[:])
            sl_q = load_slab(wq, 0)
            sl_k = load_slab(wk, 0)
            xqsb, xksb = [], []
            for i in range(NT):
                x_t = xld.tile([P, S], BF16, name=f"xq{i}", tag="x")
                nc.sync.dma_start(x_t[:], xq[ts(i, P), :])
                xqsb.append(x_t)
                x_t = xld.tile([P, S], BF16, name=f"xk{i}", tag="x")
                nc.sync.dma_start(x_t[:], xk[ts(i, P), :])
                xksb.append(x_t)
            for i in range(NT):
                nc.sync.dma_start(msk[i][:], mt[ts(i, P), :])
            nc.sync.dma_start(bo_sb[:], bob[:])

            def project(wsl, bias, ot, xtiles, pname, pool=None):
                ps = (pool or pst).tile([P, S], F32, name="ps_pj", tag="st")
                for c in range(NCH):
                    for i in range(NT):
                        nc.tensor.matmul(
                            ps[:, ts(c, CH)],
                            wsl[:, ts(i, P)],
                            xtiles[i][:, ts(c, CH)],
                            start=(i == 0),
                            stop=(i == NT - 1),
                        )
                dst = qkp.tile([P, S], BF16, name=pname, tag=pname[0])
                # two chunk evictions so the next pair's first QK matmuls (which
                # read chunk 0) unblock ~0.5us earlier
                for c in range(NCH):
                    nc.vector.tensor_scalar_add(
                        dst[:, ts(c, CH)], ps[:, ts(c, CH)], bias[:, ds(ot, 1)]
                    )
                return dst

            class ProjJob:
                """A projection whose 16 matmuls are doled out as PE filler
                between score tiles (each ~213ns matmul fills the gap the
                1038ns exp leaves per score period). Uses the 1-slot ppj
                psum pool so it never contends with the score tiles."""

                def __init__(self, wsl, bias, ot, xtiles, pname):
                    self.ps = ppj.tile([P, S], F32, name="ps_pj", tag="pj")
                    self.wsl, self.bias, self.ot = wsl, bias, ot
                    self.xtiles, self.pname = xtiles, pname
                    self.steps = [(c, i) for c in range(NCH) for i in range(NT)]
                    self.dst = None

                def step(self):
                    c, i = self.steps.pop(0)
                    nc.tensor.matmul(
                        self.ps[:, ts(c, CH)],
                        self.wsl[:, ts(i, P)],
                        self.xtiles[i][:, ts(c, CH)],
                        start=(i == 0),
                        stop=(i == NT - 1),
                    )
                    if i == NT - 1:
                        # per-chunk eviction: frees the 1-slot ppj pool for
                        # the next job sooner (slot handoff waits on all
                        # evictions of the previous tenant)
                        if self.dst is None:
                            self.dst = qkp.tile(
                                [P, S], BF16, name=self.pname, tag=self.pname[0]
                            )
                        nc.vector.tensor_scalar_add(
                            self.dst[:, ts(c, CH)],
                            self.ps[:, ts(c, CH)],
                            self.bias[:, ds(self.ot, 1)],
                        )
                    return bool(self.steps)

            class WoPass1Job:
                """First half of WO s-tile j: ps = sum_{i<4} aot[i].T-slice @
                WO-slab, evicted (with bias) to a bf16 SBUF accumulator in the
                long-idle xvp pool. Doled out as PE filler during pairs 6,7
                (which have no projection work left); the tail then only runs
                i=4..7 + the accumulator add."""

                def __init__(self, j, wosb, woacc):
                    self.ps = ppj.tile([P, D], F32, name="ps_w1", tag="pj")
                    self.j, self.wosb, self.woacc = j, wosb, woacc
                    self.steps = [(c, i) for c in range(NCH) for i in range(4)]
                    self.dst = None

                def step(self):
                    c, i = self.steps.pop(0)
                    nc.tensor.matmul(
                        self.ps[:, ts(c, CH)],
                        aot[i][:, ts(self.j, P)],
                        self.wosb[i][:, ts(c, CH)],
                        start=(i == 0),
                        stop=(i == 3),
                    )
                    if not self.steps:
                        acc = xvp.tile([P, D], BF16, name=f"wa{self.j}", tag="xv")
                        nc.vector.tensor_add(acc[:], self.ps[:], bo_sb[:])
                        self.woacc[self.j] = acc
                    return bool(self.steps)

            def head_qk(h, qt_t, kt_t, jobs):
                """scores -> exp -> mask for head h; returns the 8 E.T tiles.
                After each score tile, emits 2 filler matmuls from `jobs`."""
                prow = (h % 2) * 64
                eh = []
                for i in range(NT):
                    st_ps = pst.tile([P, S], F32, name="st", tag="st")
                    for c in range(NCH):
                        nc.tensor.matmul(
                            st_ps[:, ts(c, CH)],
                            kt_t[ds(prow, 64), ts(i, P)],
                            qt_t[ds(prow, 64), ts(c, CH)],
                            start=True,
                            stop=True,
                        )
                    e = epool.tile([P, S], BF16, name=f"e{i}", tag="e")
                    nc.scalar.activation(e[:], st_ps[:], mybir.ActivationFunctionType.Exp)
                    # masks all on GPSIMD: keeps the DVE queue free of ACT-paced
                    # work so the q/k projection evictions (which gate the next
                    # pair's QK) run as soon as their psum completes
                    if mask_on_gpsimd:
                        nc.gpsimd.tensor_mul(e[:], e[:], msk[i][:])
                    else:
                        nc.vector.tensor_mul(e[:], e[:], msk[i][:])
                    eh.append(e)
                    for _ in range(2):
                        if jobs and not jobs[0].step():
                            jobs.pop(0)
                return eh

            def head_av(h, eh, aopair):
                prow = (h % 2) * 64
                for j in range(NT):
                    av = ps_av.tile([P, P], F32, name="av", tag="av")
                    for i in range(NT):
                        nc.tensor.matmul(
                            av[:, 0:65],
                            eh[i][:, ts(j, P)],
                            vv[i][:, ds(h * 65, 65)],
                            start=(i == 0),
                            stop=(i == NT - 1),
                        )
                    rc = rpool.tile([P, 1], F32, name="rc", tag="rc")
                    nc.vector.reciprocal(rc[:], av[:, ds(64, 1)])
                    nc.vector.tensor_scalar_mul(
                        aopair[j][:, ds(prow, 64)], av[:, 0:64], rc[:]
                    )

            def transpose_pair(t, aopair):
                # all 8 [128,128]bf16 transposes fit ONE psum bank: 1 slot + 1 big
                # DVE copy instead of 8 of each. (DMA-xbar transposes measured
                # WORSE: they contend with the chained input prefetch on the
                # in-order SP queue.)
                ptb = ps_av.tile([P, S], BF16, name="ptb", tag="av")
                for j in range(NT):
                    nc.tensor.matmul(
                        ptb[:, ts(j, P)],
                        aopair[j][:],
                        ident[:],
                        is_transpose=True,
                        start=(j == 0),
                        stop=(j == NT - 1),
                        skip_group_check=True,
                    )
                nc.vector.tensor_copy(aot[t][:], ptb[:])

            # ---- V projection first (dense PE work during input DMA; AV depends
            # on all of V) ----
            # V[s, d]: stationary = x.T [i,s]-tile, moving = W.T [i,o].
            # Groups are processed 3 at a time (= pp bufs) with their i-loops
            # interleaved: the stream is paced by each wv/xv tile's DMA arrival,
            # so 3-way interleave gives PE 3x the work per arriving tile.
            # 4 groups per batch: 3 through pp (2-bank tiles) + 1 split into its
            # two 512-chunks through the ps_av slots (1 bank each, idle during
            # V-proj) -> 8 matmuls per arriving wv/xv tile instead of 6 during
            # the DMA-paced phase, and 2 batches instead of 3.
            for g0 in (0, 4):
                batch = list(range(g0, g0 + 4))
                pss = {}
                for st_ in batch:
                    nc.gpsimd.memset(
                        vv[st_].rearrange("p (g c) -> p g c", c=65)[:, :, 64:65], 1.0
                    )
                pss[batch[0]] = pst.tile([P, D], F32, name="ps_pv", tag="st")
                pss[batch[1]] = pst.tile([P, D], F32, name="ps_pv", tag="st")
                pss[batch[2]] = ppj.tile([P, D], F32, name="ps_pv", tag="pj")
                sp = batch[3]
                half = [
                    ps_av.tile([P, CH], F32, name="ps_ph", tag="av") for _ in range(NCH)
                ]
                for i in range(NT):
                    for st_ in batch[:3]:
                        for c in range(NCH):
                            nc.tensor.matmul(
                                pss[st_][:, ts(c, CH)],
                                xvsb[i][:, ts(st_, P)],
                                wvsb[i][:, ts(c, CH)],
                                start=(i == 0),
                                stop=(i == NT - 1),
                            )
                    for c in range(NCH):
                        nc.tensor.matmul(
                            half[c][:],
                            xvsb[i][:, ts(sp, P)],
                            wvsb[i][:, ts(c, CH)],
                            start=(i == 0),
                            stop=(i == NT - 1),
                        )
                for st_ in batch[:3]:
                    # scatter 16 head-blocks of 64 into 65-strided layout, + bias
                    nc.vector.tensor_add(
                        vv[st_].rearrange("p (g c) -> p g c", c=65)[:, :, 0:64],
                        pss[st_].rearrange("p (g c) -> p g c", c=64),
                        bv_sb.rearrange("p (g c) -> p g c", c=65)[:, :, 0:64],
                    )
                for c in range(NCH):
                    g0c = c * 8
                    nc.vector.tensor_add(
                        vv[sp][:, ds(g0c * 65, 8 * 65)].rearrange(
                            "p (g c) -> p g c", c=65
                        )[:, :, 0:64],
                        half[c].rearrange("p (g c) -> p g c", c=64),
                        bv_sb[:, ds(g0c * 65, 8 * 65)].rearrange(
                            "p (g c) -> p g c", c=65
                        )[:, :, 0:64],
                    )

            # ---- main loop over head pairs ----
            # static PE order per pair: QK (feeds ACT) -> next-pair projection
            # (fills PE while ACT runs the exps) -> AV(2t) -> previous pair's
            # transposes (extra PE filler before AV(2t+1)'s E is ready) -> AV(2t+1)
            def wo_stile2(j, wosb, woacc):
                # second half (i=4..7) of WO s-tile j + the pass-1 accumulator.
                # psum alternates pst/ppj so eviction of tile j overlaps the
                # matmuls of tile j+1 (scores/projections are done by now).
                pool, tag = (pst, "st") if j % 2 else (ppj, "pj")
                ps = pool.tile([P, D], F32, name="ps_wo", tag=tag)
                for c in range(NCH):
                    for i in range(4, NT):
                        nc.tensor.matmul(
                            ps[:, ts(c, CH)],
                            aot[i][:, ts(j, P)],
                            wosb[i][:, ts(c, CH)],
                            start=(i == 4),
                            stop=(i == NT - 1),
                        )
                # final add + out DMA in 512-chunks: halves the last tile's
                # serial evict->DMA latency (adds on DVE — Pool can't read PSUM)
                osb = opool.tile([P, D], F32, name="osb", tag="osb")
                for c in range(NCH):
                    nc.vector.tensor_add(
                        osb[:, ts(c, CH)], ps[:, ts(c, CH)], woacc[j][:, ts(c, CH)]
                    )
                    nc.sync.dma_start(out[ts(j, P), ts(c, CH)], osb[:, ts(c, CH)])

            # projections run two pairs ahead (qkp bufs=3 per tag: current, next,
            # next-next) so iteration 0's AVs have a full exp-latency of PE filler
            qts = {0: project(sl_q, bq_sb, 0, xqsb, "qt")}
            kts = {0: project(sl_k, bk_sb, 0, xksb, "kt")}
            qts[1] = project(load_slab(wq, 1), bq_sb, 1, xqsb, "qt")
            kts[1] = project(load_slab(wk, 1), bk_sb, 1, xksb, "kt")

            prev = None
            wosb = []
            woacc = [None] * NT
            for t in range(NT):
                qt_t, kt_t = qts.pop(t), kts.pop(t)
                aopair = [
                    aop.tile([P, P], BF16, name=f"aop{j}", tag="aop") for j in range(NT)
                ]
                # next-next pair's projections ride along as per-period PE
                # filler inside the score phases (2 matmuls per exp period):
                # q-proj fills head a's phase, k-proj fills head b's. Pairs
                # 6,7 have no projections left; they get WO pass-1 instead
                # (4 j-tiles per pair x 8 steps = the same 32 slots).
                jobs = []
                jq = jk = None
                if t + 2 < NT:
                    jq = ProjJob(load_slab(wq, t + 2), bq_sb, t + 2, xqsb, "qt")
                    jobs.append(jq)
                else:
                    jobs.extend(
                        WoPass1Job(j, wosb, woacc)
                        for j in range(4 * (t - (NT - 2)), 4 * (t - (NT - 2)) + 2)
                    )
                eh_a = head_qk(2 * t, qt_t, kt_t, jobs)
                if t + 2 < NT:
                    jk = ProjJob(load_slab(wk, t + 2), bk_sb, t + 2, xksb, "kt")
                    jobs.append(jk)
                else:
                    jobs.extend(
                        WoPass1Job(j, wosb, woacc)
                        for j in range(4 * (t - (NT - 2)) + 2, 4 * (t - (NT - 2)) + 4)
                    )
                eh_b = head_qk(2 * t + 1, qt_t, kt_t, jobs)
                while jobs:  # safety drain (steps == slots normally)
                    if not jobs[0].step():
                        jobs.pop(0)
                if t + 2 < NT:
                    qts[t + 2] = jq.dst
                    kts[t + 2] = jk.dst
                if t == 4:
                    # prefetch WO weights
                    for i in range(NT):
                        w_t = wold.tile([P, D], BF16, name=f"wo{i}", tag="wo")
                        nc.sync.dma_start(w_t[:], wo[ts(i, P), :])
                        wosb.append(w_t)
                if prev is not None:
                    transpose_pair(t - 1, prev)
                head_av(2 * t, eh_a, aopair)
                if t < NT - 1:
                    head_av(2 * t + 1, eh_b, aopair)
                else:
                    # tail: interleave the last head's AV with its transposes and
                    # the WO s-tiles so the output projection starts per-j instead
                    # of waiting for the whole pair
                    prow = 64
                    for j in range(NT + 2):
                        if j < NT:
                            av = ps_av.tile([P, P], F32, name="av", tag="av")
                            for i in range(NT):
                                nc.tensor.matmul(
                                    av[:, 0:65],
                                    eh_b[i][:, ts(j, P)],
                                    vv[i][:, ds((2 * t + 1) * 65, 65)],
                                    start=(i == 0),
                                    stop=(i == NT - 1),
                                )
                            rc = rpool.tile([P, 1], F32, name="rc", tag="rc")
                            nc.vector.reciprocal(rc[:], av[:, ds(64, 1)])
                            nc.vector.tensor_scalar_mul(
                                aopair[j][:, ds(prow, 64)], av[:, 0:64], rc[:]
                            )
                        if 1 <= j <= NT:
                            pt = ps_av.tile([P, P], BF16, name="pt", tag="av")
                            nc.tensor.transpose(pt[:], aopair[j - 1][:], ident[:])
                            nc.vector.tensor_copy(aot[t][:, ts(j - 1, P)], pt[:])
                        if j >= 2:
                            wo_stile2(j - 2, wosb, woacc)
                prev = aopair

        for _it in range(iters):
            emit_body()

    nc.compile()
    return nc


def prep_inputs(q, k, v, mask, WQ_w, WQ_b, WK_w, WK_b, WV_w, WV_b, WO_w, WO_b):
    """Build the 8 per-core input maps (host-side layout prep)."""
    f32 = np.float32

    def slabs(wt):  # [D,D] W.T -> [NT, P, D]: [t][p][i*128+f] = wt[i*128+p, t*128+f]
        return np.ascontiguousarray(
            wt.reshape(NT, P, NT, P).transpose(2, 1, 0, 3).reshape(NT, P, D)
        )

    wq_t = slabs((WQ_w.astype(f32) * 0.125).T).astype(NPBF)
    wk_t = slabs(WK_w.astype(f32).T).astype(NPBF)
    wv_t = np.ascontiguousarray(WV_w.astype(f32).T).astype(NPBF)
    wo_t = np.ascontiguousarray(WO_w.astype(f32).T).astype(NPBF)
    bq_l = np.ascontiguousarray((WQ_b.astype(f32) * 0.125).reshape(NT, P).T)
    bk_l = np.ascontiguousarray(WK_b.astype(f32).reshape(NT, P).T)
    bvb = np.zeros((P, H * 65), NPBF)
    bv_f = WV_b.astype(f32)
    for h in range(H):
        bvb[:, h * 65 : h * 65 + 64] = bv_f[h * 64 : (h + 1) * 64].astype(NPBF)[None, :]
    bob = np.ascontiguousarray(np.broadcast_to(WO_b.astype(f32), (P, D)))

    in_maps = []
    for b in range(B):
        in_maps.append(
            {
                "xq": np.ascontiguousarray(q[b].astype(f32).T).astype(NPBF),
                "xk": np.ascontiguousarray(k[b].astype(f32).T).astype(NPBF),
                "xv": np.ascontiguousarray(v[b].astype(f32).T).astype(NPBF),
                "wq": wq_t,
                "wk": wk_t,
                "wv": wv_t,
                "wo": wo_t,
                "bq": bq_l,
                "bk": bk_l,
                "bvb": bvb,
                "bob": bob,
                "mt": np.ascontiguousarray(mask[b, 0].T.astype(f32)).astype(NPBF),
            }
        )
    return in_maps


def _ensure_neuron_backend():
    # if jax was already initialized cpu-only (e.g. JAX_PLATFORMS=cpu was set
    # before this module was imported), re-discover the neuron/axon backend
    import jax

    try:
        if all(d.platform == "cpu" for d in jax.devices()):
            jax.clear_backends()
    except Exception:
        pass


def kernel(q, k, v, mask, WQ_w, WQ_b, WK_w, WK_b, WV_w, WV_b, WO_w, WO_b):
    global _NC_CACHE, LAST_RESULTS
    _ensure_neuron_backend()
    if _NC_CACHE is None:
        _NC_CACHE = build_nc()
    nc = _NC_CACHE
    in_maps = prep_inputs(
        q, k, v, mask, WQ_w, WQ_b, WK_w, WK_b, WV_w, WV_b, WO_w, WO_b
    )
    res = run_bass_kernel_spmd(nc, in_maps, core_ids=list(range(B)))
    LAST_RESULTS = res
    return np.stack([res.results[b]["out"] for b in range(B)], axis=0).astype(
        np.float32
    )



# revision 15
# speedup vs baseline: 1.1918x; 1.1789x over previous
"""Multi-head attention (B=8, S=1024, D=1024, H=16) on 8 TRN2 NeuronCores.

Sharding: pure data-parallel over batch — core b computes batch b entirely
locally (no collectives). All matmuls run in bf16 with fp32 PSUM accumulation.

Per-core dataflow (host pre-transposes inputs/weights so no on-chip input
transposes are needed):
  Q_t[d,s], K_t[d,s] projected per d-tile (scale 1/sqrt(dk) folded into
  WQ/bq on the host; WQ/WK arrive as host-prearranged column slabs so they
  stream through SBUF instead of staying resident). V[s,d] is stored with a
  ones-column interleaved per head so the attention-value matmul also
  produces softmax row sums. Per head:
    S.T[k,q] = K_t_h.T @ Q_t_h   (single K=64 matmul per 512-chunk)
    E.T = exp(S.T) * mask.T      (ACT exp from PSUM; mask mult on GPSIMD)
    psum[q, 0:65] = sum_k E.T_tile.T @ [V_h | 1]  -> out + rowsum
    attnout[q, d_h] = psum[:,0:64] * recip(psum[:,64])   (DVE)
  attnout transposed via PE -> WO projection -> + bias -> out[s,o] fp32.

Schedule: V projection first (dense PE work while inputs stream in, psum
groups 4-way interleaved over the arriving wv/xv tiles). Main loop over
head pairs: each exp period (one [128,1024] ACT exp per score tile) gets
exactly 2 filler matmuls emitted inline — the next-next pair's Q-proj
fills head a's score phase, K-proj fills head b's (ProjJob), with AV of
the previous head pulled into the residue. Pairs 6,7 have no projections
left; they run WO pass 1 (contraction over aot[0..3]) as filler into bf16
SBUF accumulators reusing the idle xv pool, so the tail only runs WO pass
2 (aot[4..7]) + accumulator add, chunked 512-wide with psum alternating
between pools. PSUM: scores 2x2 banks, proj/WO 1x2, AV+transpose 2x1.
Masks multiply on GPSIMD/Pool (SBUF only — Pool cannot touch PSUM).

Cost model: ~213us single-shot in CoreSim at 92% PE occupancy (PE busy
196.5us). Measured HW engine rates diverge from CoreSim substantially
(marginal microbenches, this session):
  PE  N=512 MM streams: ~119ns/MM stationary-reused / ~189 fresh
      (sim 213) -> HW PE is FASTER than sim; LDWEIGHTS elision on
      repeated stationaries is worth ~35% -> keep i-outer/c-inner loops.
  ACT [128,1024] exp PSUM->SBUF: ~1173ns (sim 1038) -> the 128 exps are
      a ~150us serial floor; ablation of just DMA+proj+scores+exp+mask
      measured 153us/iter = the pipeline is ACT-paced and healthy.
  POOL [128,1024] bf16 mul: ~946ns (sim 853). DVE ~0.8x sim.
  PE transpose-mode: ~460ns per 128x128 (sim ~25ns, 6.2x) -> ~33us/iter
      hidden PE cost; DMA-xbar transposes measured WORSE (SP-queue
      contention with chained input prefetch) - keep them on PE.
  DMA: 20MB/iter on 8 cores = 55us/iter = ~363GB/s/core, matches spec.
fp8(e4m3) was numerically simulated and REJECTED: quantizing any one
projection already costs 1.9-3.0% rel err vs the 2e-2 gate (bf16 base
0.33%).
Measured marginal: 272.6us (grader) / 325us (this session, cool) with
the HW exec state drifting to ~385us after an hour of sustained
benching (same binary re-measured; thermal/power-state drift) - do not
trust <10% deltas across runs spaced by many benches.
This version additionally: dedicates a pool to the WO pass-1
accumulators (they used to alias xvp, which let the in-order SP queue
couple next-iteration input DMAs to this iteration's tail), reuses
matmul stationaries via i-outer/c-inner loop orders, and emits the
output (+WO bias) in bf16 (halves out-DMA bytes; rel err 0.0041 vs
0.0036 all-f32, gate 2e-2).
"""

import os
import sys
from contextlib import ExitStack

import numpy as np

if os.environ.get("JAX_PLATFORMS") == "cpu":
    # bass execution needs the neuron/axon jax backend
    del os.environ["JAX_PLATFORMS"]

for _p in ("/opt/trn_rl_repo",):
    if _p not in sys.path and os.path.isdir(_p):
        sys.path.insert(0, _p)

import ml_dtypes

import concourse.bass as bass
import concourse.mybir as mybir
import concourse.tile as tile
from concourse import bacc
from concourse.bass import ds, ts
from concourse.bass_utils import run_bass_kernel_spmd
from concourse.masks import make_identity

BF16 = mybir.dt.bfloat16
F32 = mybir.dt.float32
NPBF = ml_dtypes.bfloat16

B, S, D, H, DK = 8, 1024, 1024, 16, 64
P = 128
NT = D // P  # 8 tiles along any 1024 dim
CH = 512  # matmul moving-dim chunk (one PSUM bank of fp32)
NCH = S // CH  # 2

MASK_ON_GPSIMD = True

LAST_RESULTS = None
_NC_CACHE = None


def build_nc(iters=1, mask_on_gpsimd=MASK_ON_GPSIMD):
    nc = bacc.Bacc("TRN2", target_bir_lowering=False, debug=False)

    xq = nc.dram_tensor("xq", [D, S], BF16, kind="ExternalInput")  # q[b].T
    xk = nc.dram_tensor("xk", [D, S], BF16, kind="ExternalInput")
    xv = nc.dram_tensor("xv", [D, S], BF16, kind="ExternalInput")
    # wq/wk: host-prearranged column slabs [t][p][i*128+f] = W.T[i*128+p, t*128+f]
    wq = nc.dram_tensor("wq", [NT, P, D], BF16, kind="ExternalInput")
    wk = nc.dram_tensor("wk", [NT, P, D], BF16, kind="ExternalInput")
    wv = nc.dram_tensor("wv", [D, D], BF16, kind="ExternalInput")  # WV_w.T
    wo = nc.dram_tensor("wo", [D, D], BF16, kind="ExternalInput")  # WO_w.T
    bq = nc.dram_tensor("bq", [P, NT], F32, kind="ExternalInput")  # WQ_b/8
    bk = nc.dram_tensor("bk", [P, NT], F32, kind="ExternalInput")
    bvb = nc.dram_tensor("bvb", [P, H * 65], BF16, kind="ExternalInput")
    bob = nc.dram_tensor("bob", [P, D], BF16, kind="ExternalInput")
    mt = nc.dram_tensor("mt", [S, S], BF16, kind="ExternalInput")  # mask[b,0].T
    out = nc.dram_tensor("out", [S, D], BF16, kind="ExternalOutput")

    with tile.TileContext(nc) as tc, ExitStack() as ctx:
        pers = ctx.enter_context(tc.tile_pool(name="pers", bufs=1))
        # xq+xk resident for the whole kernel
        xld = ctx.enter_context(tc.tile_pool(name="xld", bufs=16))
        xvp = ctx.enter_context(tc.tile_pool(name="xvp", bufs=8))
        # wv and wo each get dedicated slots: sharing one pool couples the
        # next iteration's wv DMAs to this iteration's WO tail (SP issues
        # descriptors in order, so one waiting DMA head-of-line blocks all
        # later input loads at the iteration boundary)
        wld = ctx.enter_context(tc.tile_pool(name="wld", bufs=8))
        wold = ctx.enter_context(tc.tile_pool(name="wold", bufs=8))
        wslab = ctx.enter_context(tc.tile_pool(name="wslab", bufs=3))
        # WO pass-1 accumulators get their OWN pool: when they borrowed xvp
        # slots, the next iteration's xv DMAs blocked on this iteration's
        # TAIL, and the in-order SP queue then stalled the whole next input
        # stream (no cross-iteration prefetch).
        wacc = ctx.enter_context(tc.tile_pool(name="wacc", bufs=8))
        # q/k projection outputs: only live for their head pair -> rotate
        qkp = ctx.enter_context(tc.tile_pool(name="qkp", bufs=3))
        epool = ctx.enter_context(tc.tile_pool(name="epool", bufs=16))
        aop = ctx.enter_context(tc.tile_pool(name="aop", bufs=16))
        opool = ctx.enter_context(tc.tile_pool(name="opool", bufs=2))
        rpool = ctx.enter_context(tc.tile_pool(name="rpool", bufs=8))
        # psum: scores get their own 2-slot pool (2 banks each) so the
        # in-loop projection (1-slot ppj pool) never steals a slot the
        # exp pipeline is about to recycle; ps_av keeps 2 one-bank slots
        # for AV accumulation + attnout transposes. 2*2 + 1*2 + 2*1 = 8.
        pst = ctx.enter_context(tc.tile_pool(name="pst", bufs=2, space="PSUM"))
        ppj = ctx.enter_context(tc.tile_pool(name="ppj", bufs=1, space="PSUM"))
        ps_av = ctx.enter_context(tc.tile_pool(name="ps_av", bufs=2, space="PSUM"))

        # ---- persistent tiles ----
        vv = [
            pers.tile([P, H * 65], BF16, name=f"vv{t}", tag=f"vv{t}")
            for t in range(NT)
        ]
        msk = [pers.tile([P, S], BF16, name=f"mk{t}", tag=f"mk{t}") for t in range(NT)]
        aot = [pers.tile([P, S], BF16, name=f"at{t}", tag=f"at{t}") for t in range(NT)]
        ident = pers.tile([P, P], BF16, name="ident", tag="ident")
        bq_sb = pers.tile([P, NT], F32, name="bq_sb", tag="bq_sb")
        bk_sb = pers.tile([P, NT], F32, name="bk_sb", tag="bk_sb")
        bv_sb = pers.tile([P, H * 65], BF16, name="bv_sb", tag="bv_sb")
        bo_sb = pers.tile([P, D], BF16, name="bo_sb", tag="bo_sb")

        make_identity(nc, ident)

        def emit_body():
            nc.sync.dma_start(bq_sb[:], bq[:])
            nc.sync.dma_start(bk_sb[:], bk[:])

            def load_slab(wdram, ot):
                wsl = wslab.tile([P, D], BF16, name="wsl", tag="ws")
                nc.sync.dma_start(wsl[:], wdram[ot])
                return wsl

            # ---- input DMAs: V-path first — V-proj is ~2x denser PE work per DMA
            # byte than the Q/K path, so it best fills the DMA-paced startup.
            # x/slab/mask DMAs land while V-proj computes. ----
            wvsb = []
            xvsb = []
            for i in range(NT):
                w_t = wld.tile([P, D], BF16, name=f"wv{i}", tag="w")
                nc.sync.dma_start(w_t[:], wv[ts(i, P), :])
                wvsb.append(w_t)
                x_t = xvp.tile([P, S], BF16, name=f"xv{i}", tag="xv")
                nc.sync.dma_start(x_t[:], xv[ts(i, P), :])
                xvsb.append(x_t)
                if i == 0:
                    nc.sync.dma_start(bv_sb[:], bvb[:])
            sl_q = load_slab(wq, 0)
            sl_k = load_slab(wk, 0)
            xqsb, xksb = [], []
            for i in range(NT):
                x_t = xld.tile([P, S], BF16, name=f"xq{i}", tag="x")
                nc.sync.dma_start(x_t[:], xq[ts(i, P), :])
                xqsb.append(x_t)
                x_t = xld.tile([P, S], BF16, name=f"xk{i}", tag="x")
                nc.sync.dma_start(x_t[:], xk[ts(i, P), :])
                xksb.append(x_t)
            for i in range(NT):
                nc.sync.dma_start(msk[i][:], mt[ts(i, P), :])
            nc.sync.dma_start(bo_sb[:], bob[:])

            def project(wsl, bias, ot, xtiles, pname, pool=None):
                ps = (pool or pst).tile([P, S], F32, name="ps_pj", tag="st")
                # i outer / c inner: consecutive matmuls share the stationary
                # operand, so the redundant LDWEIGHTS is elided (HW ~130ns/MM
                # vs ~190 with a fresh stationary every MM)
                for i in range(NT):
                    for c in range(NCH):
                        nc.tensor.matmul(
                            ps[:, ts(c, CH)],
                            wsl[:, ts(i, P)],
                            xtiles[i][:, ts(c, CH)],
                            start=(i == 0),
                            stop=(i == NT - 1),
                        )
                dst = qkp.tile([P, S], BF16, name=pname, tag=pname[0])
                # two chunk evictions so the next pair's first QK matmuls (which
                # read chunk 0) unblock ~0.5us earlier
                for c in range(NCH):
                    nc.vector.tensor_scalar_add(
                        dst[:, ts(c, CH)], ps[:, ts(c, CH)], bias[:, ds(ot, 1)]
                    )
                return dst

            class ProjJob:
                """A projection whose 16 matmuls are doled out as PE filler
                between score tiles (each ~213ns matmul fills the gap the
                1038ns exp leaves per score period). Uses the 1-slot ppj
                psum pool so it never contends with the score tiles."""

                def __init__(self, wsl, bias, ot, xtiles, pname):
                    self.ps = ppj.tile([P, S], F32, name="ps_pj", tag="pj")
                    self.wsl, self.bias, self.ot = wsl, bias, ot
                    self.xtiles, self.pname = xtiles, pname
                    # i outer / c inner: stationary reuse between the two
                    # chunks of each i (LDWEIGHTS elision)
                    self.steps = [(c, i) for i in range(NT) for c in range(NCH)]
                    self.dst = None

                def step(self):
                    c, i = self.steps.pop(0)
                    nc.tensor.matmul(
                        self.ps[:, ts(c, CH)],
                        self.wsl[:, ts(i, P)],
                        self.xtiles[i][:, ts(c, CH)],
                        start=(i == 0),
                        stop=(i == NT - 1),
                    )
                    if i == NT - 1:
                        # per-chunk eviction: frees the 1-slot ppj pool for
                        # the next job sooner (slot handoff waits on all
                        # evictions of the previous tenant)
                        if self.dst is None:
                            self.dst = qkp.tile(
                                [P, S], BF16, name=self.pname, tag=self.pname[0]
                            )
                        nc.vector.tensor_scalar_add(
                            self.dst[:, ts(c, CH)],
                            self.ps[:, ts(c, CH)],
                            self.bias[:, ds(self.ot, 1)],
                        )
                    return bool(self.steps)

            class WoPass1Job:
                """First half of WO s-tile j: ps = sum_{i<4} aot[i].T-slice @
                WO-slab, evicted (with bias) to a bf16 SBUF accumulator in the
                long-idle xvp pool. Doled out as PE filler during pairs 6,7
                (which have no projection work left); the tail then only runs
                i=4..7 + the accumulator add."""

                def __init__(self, j, wosb, woacc):
                    self.ps = ppj.tile([P, D], F32, name="ps_w1", tag="pj")
                    self.j, self.wosb, self.woacc = j, wosb, woacc
                    self.steps = [(c, i) for i in range(4) for c in range(NCH)]
                    self.dst = None

                def step(self):
                    c, i = self.steps.pop(0)
                    nc.tensor.matmul(
                        self.ps[:, ts(c, CH)],
                        aot[i][:, ts(self.j, P)],
                        self.wosb[i][:, ts(c, CH)],
                        start=(i == 0),
                        stop=(i == 3),
                    )
                    if not self.steps:
                        acc = wacc.tile([P, D], BF16, name=f"wa{self.j}", tag="wa")
                        nc.vector.tensor_add(acc[:], self.ps[:], bo_sb[:])
                        self.woacc[self.j] = acc
                    return bool(self.steps)

            def head_qk(h, qt_t, kt_t, jobs):
                """scores -> exp -> mask for head h; returns the 8 E.T tiles.
                After each score tile, emits 2 filler matmuls from `jobs`."""
                prow = (h % 2) * 64
                eh = []
                for i in range(NT):
                    st_ps = pst.tile([P, S], F32, name="st", tag="st")
                    for c in range(NCH):
                        nc.tensor.matmul(
                            st_ps[:, ts(c, CH)],
                            kt_t[ds(prow, 64), ts(i, P)],
                            qt_t[ds(prow, 64), ts(c, CH)],
                            start=True,
                            stop=True,
                        )
                    e = epool.tile([P, S], BF16, name=f"e{i}", tag="e")
                    nc.scalar.activation(e[:], st_ps[:], mybir.ActivationFunctionType.Exp)
                    # masks all on GPSIMD: keeps the DVE queue free of ACT-paced
                    # work so the q/k projection evictions (which gate the next
                    # pair's QK) run as soon as their psum completes
                    if mask_on_gpsimd:
                        nc.gpsimd.tensor_mul(e[:], e[:], msk[i][:])
                    else:
                        nc.vector.tensor_mul(e[:], e[:], msk[i][:])
                    eh.append(e)
                    for _ in range(2):
                        if jobs and not jobs[0].step():
                            jobs.pop(0)
                return eh

            def head_av(h, eh, aopair):
                prow = (h % 2) * 64
                for j in range(NT):
                    av = ps_av.tile([P, P], F32, name="av", tag="av")
                    for i in range(NT):
                        nc.tensor.matmul(
                            av[:, 0:65],
                            eh[i][:, ts(j, P)],
                            vv[i][:, ds(h * 65, 65)],
                            start=(i == 0),
                            stop=(i == NT - 1),
                        )
                    rc = rpool.tile([P, 1], F32, name="rc", tag="rc")
                    nc.vector.reciprocal(rc[:], av[:, ds(64, 1)])
                    nc.vector.tensor_scalar_mul(
                        aopair[j][:, ds(prow, 64)], av[:, 0:64], rc[:]
                    )

            def transpose_pair(t, aopair):
                # all 8 [128,128]bf16 transposes fit ONE psum bank: 1 slot + 1 big
                # DVE copy instead of 8 of each. (DMA-xbar transposes measured
                # WORSE: they contend with the chained input prefetch on the
                # in-order SP queue.)
                ptb = ps_av.tile([P, S], BF16, name="ptb", tag="av")
                for j in range(NT):
                    nc.tensor.matmul(
                        ptb[:, ts(j, P)],
                        aopair[j][:],
                        ident[:],
                        is_transpose=True,
                        start=(j == 0),
                        stop=(j == NT - 1),
                        skip_group_check=True,
                    )
                nc.vector.tensor_copy(aot[t][:], ptb[:])

            # ---- V projection first (dense PE work during input DMA; AV depends
            # on all of V) ----
            # V[s, d]: stationary = x.T [i,s]-tile, moving = W.T [i,o].
            # Groups are processed 3 at a time (= pp bufs) with their i-loops
            # interleaved: the stream is paced by each wv/xv tile's DMA arrival,
            # so 3-way interleave gives PE 3x the work per arriving tile.
            # 4 groups per batch: 3 through pp (2-bank tiles) + 1 split into its
            # two 512-chunks through the ps_av slots (1 bank each, idle during
            # V-proj) -> 8 matmuls per arriving wv/xv tile instead of 6 during
            # the DMA-paced phase, and 2 batches instead of 3.
            for g0 in (0, 4):
                batch = list(range(g0, g0 + 4))
                pss = {}
                for st_ in batch:
                    nc.gpsimd.memset(
                        vv[st_].rearrange("p (g c) -> p g c", c=65)[:, :, 64:65], 1.0
                    )
                pss[batch[0]] = pst.tile([P, D], F32, name="ps_pv", tag="st")
                pss[batch[1]] = pst.tile([P, D], F32, name="ps_pv", tag="st")
                pss[batch[2]] = ppj.tile([P, D], F32, name="ps_pv", tag="pj")
                sp = batch[3]
                half = [
                    ps_av.tile([P, CH], F32, name="ps_ph", tag="av") for _ in range(NCH)
                ]
                for i in range(NT):
                    for st_ in batch[:3]:
                        for c in range(NCH):
                            nc.tensor.matmul(
                                pss[st_][:, ts(c, CH)],
                                xvsb[i][:, ts(st_, P)],
                                wvsb[i][:, ts(c, CH)],
                                start=(i == 0),
                                stop=(i == NT - 1),
                            )
                    for c in range(NCH):
                        nc.tensor.matmul(
                            half[c][:],
                            xvsb[i][:, ts(sp, P)],
                            wvsb[i][:, ts(c, CH)],
                            start=(i == 0),
                            stop=(i == NT - 1),
                        )
                for st_ in batch[:3]:
                    # scatter 16 head-blocks of 64 into 65-strided layout, + bias
                    nc.vector.tensor_add(
                        vv[st_].rearrange("p (g c) -> p g c", c=65)[:, :, 0:64],
                        pss[st_].rearrange("p (g c) -> p g c", c=64),
                        bv_sb.rearrange("p (g c) -> p g c", c=65)[:, :, 0:64],
                    )
                for c in range(NCH):
                    g0c = c * 8
                    nc.vector.tensor_add(
                        vv[sp][:, ds(g0c * 65, 8 * 65)].rearrange(
                            "p (g c) -> p g c", c=65
                        )[:, :, 0:64],
                        half[c].rearrange("p (g c) -> p g c", c=64),
                        bv_sb[:, ds(g0c * 65, 8 * 65)].rearrange(
                            "p (g c) -> p g c", c=65
                        )[:, :, 0:64],
                    )

            # ---- main loop over head pairs ----
            # static PE order per pair: QK (feeds ACT) -> next-pair projection
            # (fills PE while ACT runs the exps) -> AV(2t) -> previous pair's
            # transposes (extra PE filler before AV(2t+1)'s E is ready) -> AV(2t+1)
            def wo_stile2(j, wosb, woacc):
                # second half (i=4..7) of WO s-tile j + the pass-1 accumulator.
                # psum alternates pst/ppj so eviction of tile j overlaps the
                # matmuls of tile j+1 (scores/projections are done by now).
                pool, tag = (pst, "st") if j % 2 else (ppj, "pj")
                ps = pool.tile([P, D], F32, name="ps_wo", tag=tag)
                for i in range(4, NT):
                    for c in range(NCH):
                        nc.tensor.matmul(
                            ps[:, ts(c, CH)],
                            aot[i][:, ts(j, P)],
                            wosb[i][:, ts(c, CH)],
                            start=(i == 4),
                            stop=(i == NT - 1),
                        )
                # final add + out DMA in 512-chunks: halves the last tile's
                # serial evict->DMA latency (adds on DVE — Pool can't read PSUM)
                osb = opool.tile([P, D], BF16, name="osb", tag="osb")
                for c in range(NCH):
                    nc.vector.tensor_add(
                        osb[:, ts(c, CH)], ps[:, ts(c, CH)], woacc[j][:, ts(c, CH)]
                    )
                    nc.sync.dma_start(out[ts(j, P), ts(c, CH)], osb[:, ts(c, CH)])

            # projections run two pairs ahead (qkp bufs=3 per tag: current, next,
            # next-next) so iteration 0's AVs have a full exp-latency of PE filler
            qts = {0: project(sl_q, bq_sb, 0, xqsb, "qt")}
            kts = {0: project(sl_k, bk_sb, 0, xksb, "kt")}
            qts[1] = project(load_slab(wq, 1), bq_sb, 1, xqsb, "qt")
            kts[1] = project(load_slab(wk, 1), bk_sb, 1, xksb, "kt")

            prev = None
            wosb = []
            woacc = [None] * NT
            for t in range(NT):
                qt_t, kt_t = qts.pop(t), kts.pop(t)
                aopair = [
                    aop.tile([P, P], BF16, name=f"aop{j}", tag="aop") for j in range(NT)
                ]
                # next-next pair's projections ride along as per-period PE
                # filler inside the score phases (2 matmuls per exp period):
                # q-proj fills head a's phase, k-proj fills head b's. Pairs
                # 6,7 have no projections left; they get WO pass-1 instead
                # (4 j-tiles per pair x 8 steps = the same 32 slots).
                jobs = []
                jq = jk = None
                if t + 2 < NT:
                    jq = ProjJob(load_slab(wq, t + 2), bq_sb, t + 2, xqsb, "qt")
                    jobs.append(jq)
                else:
                    jobs.extend(
                        WoPass1Job(j, wosb, woacc)
                        for j in range(4 * (t - (NT - 2)), 4 * (t - (NT - 2)) + 2)
                    )
                eh_a = head_qk(2 * t, qt_t, kt_t, jobs)
                if t + 2 < NT:
                    jk = ProjJob(load_slab(wk, t + 2), bk_sb, t + 2, xksb, "kt")
                    jobs.append(jk)
                else:
                    jobs.extend(
                        WoPass1Job(j, wosb, woacc)
                        for j in range(4 * (t - (NT - 2)) + 2, 4 * (t - (NT - 2)) + 4)
                    )
                eh_b = head_qk(2 * t + 1, qt_t, kt_t, jobs)
                while jobs:  # safety drain (steps == slots normally)
                    if not jobs[0].step():
                        jobs.pop(0)
                if t + 2 < NT:
                    qts[t + 2] = jq.dst
                    kts[t + 2] = jk.dst
                if t == 4:
                    # prefetch WO weights
                    for i in range(NT):
                        w_t = wold.tile([P, D], BF16, name=f"wo{i}", tag="wo")
                        nc.sync.dma_start(w_t[:], wo[ts(i, P), :])
                        wosb.append(w_t)
                if prev is not None:
                    transpose_pair(t - 1, prev)
                head_av(2 * t, eh_a, aopair)
                if t < NT - 1:
                    head_av(2 * t + 1, eh_b, aopair)
                else:
                    # tail: interleave the last head's AV with its transposes and
                    # the WO s-tiles so the output projection starts per-j instead
                    # of waiting for the whole pair
                    prow = 64
                    for j in range(NT + 2):
                        if j < NT:
                            av = ps_av.tile([P, P], F32, name="av", tag="av")
                            for i in range(NT):
                                nc.tensor.matmul(
                                    av[:, 0:65],
                                    eh_b[i][:, ts(j, P)],
                                    vv[i][:, ds((2 * t + 1) * 65, 65)],
                                    start=(i == 0),
                                    stop=(i == NT - 1),
                                )
                            rc = rpool.tile([P, 1], F32, name="rc", tag="rc")
                            nc.vector.reciprocal(rc[:], av[:, ds(64, 1)])
                            nc.vector.tensor_scalar_mul(
                                aopair[j][:, ds(prow, 64)], av[:, 0:64], rc[:]
                            )
                        if 1 <= j <= NT:
                            pt = ps_av.tile([P, P], BF16, name="pt", tag="av")
                            nc.tensor.transpose(pt[:], aopair[j - 1][:], ident[:])
                            nc.vector.tensor_copy(aot[t][:, ts(j - 1, P)], pt[:])
                        if j >= 2:
                            wo_stile2(j - 2, wosb, woacc)
                prev = aopair

        for _it in range(iters):
            emit_body()

    nc.compile()
    return nc


def prep_inputs(q, k, v, mask, WQ_w, WQ_b, WK_w, WK_b, WV_w, WV_b, WO_w, WO_b):
    """Build the 8 per-core input maps (host-side layout prep)."""
    f32 = np.float32

    def slabs(wt):  # [D,D] W.T -> [NT, P, D]: [t][p][i*128+f] = wt[i*128+p, t*128+f]
        return np.ascontiguousarray(
            wt.reshape(NT, P, NT, P).transpose(2, 1, 0, 3).reshape(NT, P, D)
        )

    wq_t = slabs((WQ_w.astype(f32) * 0.125).T).astype(NPBF)
    wk_t = slabs(WK_w.astype(f32).T).astype(NPBF)
    wv_t = np.ascontiguousarray(WV_w.astype(f32).T).astype(NPBF)
    wo_t = np.ascontiguousarray(WO_w.astype(f32).T).astype(NPBF)
    bq_l = np.ascontiguousarray((WQ_b.astype(f32) * 0.125).reshape(NT, P).T)
    bk_l = np.ascontiguousarray(WK_b.astype(f32).reshape(NT, P).T)
    bvb = np.zeros((P, H * 65), NPBF)
    bv_f = WV_b.astype(f32)
    for h in range(H):
        bvb[:, h * 65 : h * 65 + 64] = bv_f[h * 64 : (h + 1) * 64].astype(NPBF)[None, :]
    bob = np.ascontiguousarray(np.broadcast_to(WO_b.astype(f32), (P, D))).astype(NPBF)

    in_maps = []
    for b in range(B):
        in_maps.append(
            {
                "xq": np.ascontiguousarray(q[b].astype(f32).T).astype(NPBF),
                "xk": np.ascontiguousarray(k[b].astype(f32).T).astype(NPBF),
                "xv": np.ascontiguousarray(v[b].astype(f32).T).astype(NPBF),
                "wq": wq_t,
                "wk": wk_t,
                "wv": wv_t,
                "wo": wo_t,
                "bq": bq_l,
                "bk": bk_l,
                "bvb": bvb,
                "bob": bob,
                "mt": np.ascontiguousarray(mask[b, 0].T.astype(f32)).astype(NPBF),
            }
        )
    return in_maps


def _ensure_neuron_backend():
    # if jax was already initialized cpu-only (e.g. JAX_PLATFORMS=cpu was set
    # before this module was imported), re-discover the neuron/axon backend
    import jax

    try:
        if all(d.platform == "cpu" for d in jax.devices()):
            jax.clear_backends()
    except Exception:
        pass


def kernel(q, k, v, mask, WQ_w, WQ_b, WK_w, WK_b, WV_w, WV_b, WO_w, WO_b):
    global _NC_CACHE, LAST_RESULTS
    _ensure_neuron_backend()
    if _NC_CACHE is None:
        _NC_CACHE = build_nc()
    nc = _NC_CACHE
    in_maps = prep_inputs(
        q, k, v, mask, WQ_w, WQ_b, WK_w, WK_b, WV_w, WV_b, WO_w, WO_b
    )
    res = run_bass_kernel_spmd(nc, in_maps, core_ids=list(range(B)))
    LAST_RESULTS = res
    return np.stack([res.results[b]["out"] for b in range(B)], axis=0).astype(
        np.float32
    )



# revision 20
# speedup vs baseline: 1.3334x; 1.1188x over previous
"""Multi-head attention (B=8, S=1024, D=1024, H=16) on 8 TRN2 NeuronCores.

Sharding: pure data-parallel over batch — core b computes batch b entirely
locally (no collectives). All matmuls run in bf16 with fp32 PSUM accumulation.

Per-core dataflow (host pre-transposes inputs/weights so no on-chip input
transposes are needed):
  Q_t[d,s], K_t[d,s] projected per d-tile (scale 1/sqrt(dk) folded into
  WQ/bq on the host; WQ/WK arrive as host-prearranged column slabs so they
  stream through SBUF instead of staying resident). V[s,d] is stored with a
  ones-column interleaved per head so the attention-value matmul also
  produces softmax row sums. Per head:
    S.T[k,q] = K_t_h.T @ Q_t_h   (single K=64 matmul per 512-chunk)
    E.T = exp(S.T) * mask.T      (ACT exp from PSUM; mask mult on GPSIMD)
    psum[q, 0:65] = sum_k E.T_tile.T @ [V_h | 1]  -> out + rowsum
    attnout[q, d_h] = psum[:,0:64] * recip(psum[:,64])   (DVE)
  attnout transposed via PE -> WO projection -> + bias -> out[s,o] fp32.

Schedule: V projection first (dense PE work while inputs stream in, psum
groups 4-way interleaved over the arriving wv/xv tiles). Main loop over
head pairs: each exp period (one [128,1024] ACT exp per score tile) gets
exactly 2 filler matmuls emitted inline — the next-next pair's Q-proj
fills head a's score phase, K-proj fills head b's (ProjJob), with AV of
the previous head pulled into the residue. Pairs 6,7 have no projections
left; they run WO pass 1 (contraction over aot[0..3]) as filler into bf16
SBUF accumulators reusing the idle xv pool, so the tail only runs WO pass
2 (aot[4..7]) + accumulator add, chunked 512-wide with psum alternating
between pools. PSUM: scores 2x2 banks, proj/WO 1x2, AV+transpose 2x1.
Masks multiply on GPSIMD/Pool (SBUF only — Pool cannot touch PSUM).

Cost model: ~213us single-shot in CoreSim at 92% PE occupancy (PE busy
196.5us). Measured HW engine rates diverge from CoreSim substantially
(marginal microbenches, this session):
  PE  N=512 MM streams: ~119ns/MM stationary-reused / ~189 fresh
      (sim 213) -> HW PE is FASTER than sim; LDWEIGHTS elision on
      repeated stationaries is worth ~35% -> keep i-outer/c-inner loops.
  ACT [128,1024] exp PSUM->SBUF: ~1173ns (sim 1038) -> the 128 exps are
      a ~150us serial floor; ablation of just DMA+proj+scores+exp+mask
      measured 153us/iter = the pipeline is ACT-paced and healthy.
  POOL [128,1024] bf16 mul: ~946ns (sim 853). DVE ~0.8x sim.
  PE transpose-mode: ~460ns per 128x128 (sim ~25ns, 6.2x) -> ~33us/iter
      hidden PE cost; DMA-xbar transposes measured WORSE (SP-queue
      contention with chained input prefetch) - keep them on PE.
  DMA: 20MB/iter on 8 cores = 55us/iter = ~363GB/s/core, matches spec.
fp8(e4m3) was numerically simulated and REJECTED: quantizing any one
projection already costs 1.9-3.0% rel err vs the 2e-2 gate (bf16 base
0.33%).
Measured marginal: 272.6us (grader) / 325us (this session, cool) with
the HW exec state drifting to ~385us after an hour of sustained
benching (same binary re-measured; thermal/power-state drift) - do not
trust <10% deltas across runs spaced by many benches.
This version additionally: dedicates a pool to the WO pass-1
accumulators (they used to alias xvp, which let the in-order SP queue
couple next-iteration input DMAs to this iteration's tail), reuses
matmul stationaries via i-outer/c-inner loop orders, and emits the
output (+WO bias) in bf16 (halves out-DMA bytes; rel err 0.0041 vs
0.0036 all-f32, gate 2e-2).
"""

import os
import sys
from contextlib import ExitStack

import numpy as np

if os.environ.get("JAX_PLATFORMS") == "cpu":
    # bass execution needs the neuron/axon jax backend
    del os.environ["JAX_PLATFORMS"]

for _p in ("/opt/trn_rl_repo",):
    if _p not in sys.path and os.path.isdir(_p):
        sys.path.insert(0, _p)

import ml_dtypes

import concourse.bass as bass
import concourse.mybir as mybir
import concourse.tile as tile
from concourse import bacc
from concourse.bass import ds, ts
from concourse.bass_utils import run_bass_kernel_spmd
from concourse.masks import make_identity

BF16 = mybir.dt.bfloat16
F32 = mybir.dt.float32
NPBF = ml_dtypes.bfloat16

B, S, D, H, DK = 8, 1024, 1024, 16, 64
P = 128
NT = D // P  # 8 tiles along any 1024 dim
CH = 512  # matmul moving-dim chunk (one PSUM bank of fp32)
NCH = S // CH  # 2

MASK_ON_GPSIMD = True

LAST_RESULTS = None
_NC_CACHE = None


def build_nc(iters=1, mask_on_gpsimd=MASK_ON_GPSIMD):
    nc = bacc.Bacc("TRN2", target_bir_lowering=False, debug=False)

    xq = nc.dram_tensor("xq", [D, S], BF16, kind="ExternalInput")  # q[b].T
    xk = nc.dram_tensor("xk", [D, S], BF16, kind="ExternalInput")
    xv = nc.dram_tensor("xv", [D, S], BF16, kind="ExternalInput")
    # wq/wk: host-prearranged column slabs [t][p][i*128+f] = W.T[i*128+p, t*128+f]
    wq = nc.dram_tensor("wq", [NT, P, D], BF16, kind="ExternalInput")
    wk = nc.dram_tensor("wk", [NT, P, D], BF16, kind="ExternalInput")
    wv = nc.dram_tensor("wv", [D, D], BF16, kind="ExternalInput")  # WV_w.T
    wo = nc.dram_tensor("wo", [D, D], BF16, kind="ExternalInput")  # WO_w.T
    bq = nc.dram_tensor("bq", [P, NT], F32, kind="ExternalInput")  # WQ_b/8
    bk = nc.dram_tensor("bk", [P, NT], F32, kind="ExternalInput")
    bvb = nc.dram_tensor("bvb", [P, H * 65], BF16, kind="ExternalInput")
    bob = nc.dram_tensor("bob", [P, D], BF16, kind="ExternalInput")
    mt = nc.dram_tensor("mt", [S, S], BF16, kind="ExternalInput")  # mask[b,0].T
    out = nc.dram_tensor("out", [S, D], BF16, kind="ExternalOutput")

    with tile.TileContext(nc) as tc, ExitStack() as ctx:
        pers = ctx.enter_context(tc.tile_pool(name="pers", bufs=1))
        # xq+xk resident for the whole kernel
        xld = ctx.enter_context(tc.tile_pool(name="xld", bufs=16))
        xvp = ctx.enter_context(tc.tile_pool(name="xvp", bufs=8))
        # wv and wo each get dedicated slots: sharing one pool couples the
        # next iteration's wv DMAs to this iteration's WO tail (SP issues
        # descriptors in order, so one waiting DMA head-of-line blocks all
        # later input loads at the iteration boundary)
        wld = ctx.enter_context(tc.tile_pool(name="wld", bufs=8))
        wold = ctx.enter_context(tc.tile_pool(name="wold", bufs=8))
        wslab = ctx.enter_context(tc.tile_pool(name="wslab", bufs=3))
        # WO pass-1 accumulators get their OWN pool: when they borrowed xvp
        # slots, the next iteration's xv DMAs blocked on this iteration's
        # TAIL, and the in-order SP queue then stalled the whole next input
        # stream (no cross-iteration prefetch).
        wacc = ctx.enter_context(tc.tile_pool(name="wacc", bufs=8))
        # q/k projection outputs: only live for their head pair -> rotate
        qkp = ctx.enter_context(tc.tile_pool(name="qkp", bufs=3))
        epool = ctx.enter_context(tc.tile_pool(name="epool", bufs=16))
        aop = ctx.enter_context(tc.tile_pool(name="aop", bufs=16))
        opool = ctx.enter_context(tc.tile_pool(name="opool", bufs=2))
        rpool = ctx.enter_context(tc.tile_pool(name="rpool", bufs=8))
        # psum: scores get their own 2-slot pool (2 banks each) so the
        # in-loop projection (1-slot ppj pool) never steals a slot the
        # exp pipeline is about to recycle; ps_av keeps 2 one-bank slots
        # for AV accumulation + attnout transposes. 2*2 + 1*2 + 2*1 = 8.
        pst = ctx.enter_context(tc.tile_pool(name="pst", bufs=2, space="PSUM"))
        ppj = ctx.enter_context(tc.tile_pool(name="ppj", bufs=1, space="PSUM"))
        ps_av = ctx.enter_context(tc.tile_pool(name="ps_av", bufs=2, space="PSUM"))

        # ---- persistent tiles ----
        vv = [
            pers.tile([P, H * 65], BF16, name=f"vv{t}", tag=f"vv{t}")
            for t in range(NT)
        ]
        msk = [pers.tile([P, S], BF16, name=f"mk{t}", tag=f"mk{t}") for t in range(NT)]
        aot = [pers.tile([P, S], BF16, name=f"at{t}", tag=f"at{t}") for t in range(NT)]
        ident = pers.tile([P, P], BF16, name="ident", tag="ident")
        bq_sb = pers.tile([P, NT], F32, name="bq_sb", tag="bq_sb")
        bk_sb = pers.tile([P, NT], F32, name="bk_sb", tag="bk_sb")
        bv_sb = pers.tile([P, H * 65], BF16, name="bv_sb", tag="bv_sb")
        bo_sb = pers.tile([P, D], BF16, name="bo_sb", tag="bo_sb")

        make_identity(nc, ident)

        def emit_body():
            nc.sync.dma_start(bq_sb[:], bq[:])
            nc.sync.dma_start(bk_sb[:], bk[:])

            def load_slab(wdram, ot):
                wsl = wslab.tile([P, D], BF16, name="wsl", tag="ws")
                nc.sync.dma_start(wsl[:], wdram[ot])
                return wsl

            # ---- input DMAs: Q/K path FIRST so pair-0 scores (which feed the
            # ~150us serial ACT exp chain) start as early as possible; the V
            # path streams while pair 0 runs, and the V projection rides as
            # per-period filler jobs inside pairs 0-2 instead of a dedicated
            # ~25us ACT-idle phase up front. ----
            for t in range(NT):
                nc.gpsimd.memset(
                    vv[t].rearrange("p (g c) -> p g c", c=65)[:, :, 64:65], 1.0
                )
            sl_q = load_slab(wq, 0)
            sl_k = load_slab(wk, 0)
            xqsb, xksb = [], []
            for i in range(NT):
                x_t = xld.tile([P, S], BF16, name=f"xq{i}", tag="x")
                nc.sync.dma_start(x_t[:], xq[ts(i, P), :])
                xqsb.append(x_t)
                x_t = xld.tile([P, S], BF16, name=f"xk{i}", tag="x")
                nc.sync.dma_start(x_t[:], xk[ts(i, P), :])
                xksb.append(x_t)
            for i in range(NT):
                nc.sync.dma_start(msk[i][:], mt[ts(i, P), :])
            wvsb = []
            xvsb = []
            for i in range(NT):
                w_t = wld.tile([P, D], BF16, name=f"wv{i}", tag="w")
                nc.sync.dma_start(w_t[:], wv[ts(i, P), :])
                wvsb.append(w_t)
                x_t = xvp.tile([P, S], BF16, name=f"xv{i}", tag="xv")
                nc.sync.dma_start(x_t[:], xv[ts(i, P), :])
                xvsb.append(x_t)
                if i == 0:
                    nc.sync.dma_start(bv_sb[:], bvb[:])
            nc.sync.dma_start(bo_sb[:], bob[:])

            def project(wsl, bias, ot, xtiles, pname, pool=None):
                ps = (pool or pst).tile([P, S], F32, name="ps_pj", tag="st")
                # i outer / c inner: consecutive matmuls share the stationary
                # operand, so the redundant LDWEIGHTS is elided (HW ~130ns/MM
                # vs ~190 with a fresh stationary every MM)
                for i in range(NT):
                    for c in range(NCH):
                        nc.tensor.matmul(
                            ps[:, ts(c, CH)],
                            wsl[:, ts(i, P)],
                            xtiles[i][:, ts(c, CH)],
                            start=(i == 0),
                            stop=(i == NT - 1),
                        )
                dst = qkp.tile([P, S], BF16, name=pname, tag=pname[0])
                # two chunk evictions so the next pair's first QK matmuls (which
                # read chunk 0) unblock ~0.5us earlier
                for c in range(NCH):
                    nc.vector.tensor_scalar_add(
                        dst[:, ts(c, CH)], ps[:, ts(c, CH)], bias[:, ds(ot, 1)]
                    )
                return dst

            class ProjJob:
                """A projection whose 16 matmuls are doled out as PE filler
                between score tiles (each ~213ns matmul fills the gap the
                1038ns exp leaves per score period). Uses the 1-slot ppj
                psum pool so it never contends with the score tiles."""

                def __init__(self, wsl, bias, ot, xtiles, pname):
                    self.ps = ppj.tile([P, S], F32, name="ps_pj", tag="pj")
                    self.wsl, self.bias, self.ot = wsl, bias, ot
                    self.xtiles, self.pname = xtiles, pname
                    # i outer / c inner: stationary reuse between the two
                    # chunks of each i (LDWEIGHTS elision)
                    self.steps = [(c, i) for i in range(NT) for c in range(NCH)]
                    self.dst = None

                def step(self):
                    c, i = self.steps.pop(0)
                    nc.tensor.matmul(
                        self.ps[:, ts(c, CH)],
                        self.wsl[:, ts(i, P)],
                        self.xtiles[i][:, ts(c, CH)],
                        start=(i == 0),
                        stop=(i == NT - 1),
                    )
                    if i == NT - 1:
                        # per-chunk eviction: frees the 1-slot ppj pool for
                        # the next job sooner (slot handoff waits on all
                        # evictions of the previous tenant)
                        if self.dst is None:
                            self.dst = qkp.tile(
                                [P, S], BF16, name=self.pname, tag=self.pname[0]
                            )
                        nc.vector.tensor_scalar_add(
                            self.dst[:, ts(c, CH)],
                            self.ps[:, ts(c, CH)],
                            self.bias[:, ds(self.ot, 1)],
                        )
                    return bool(self.steps)

            class WoPass1Job:
                """First half of WO s-tile j: ps = sum_{i<4} aot[i].T-slice @
                WO-slab, evicted (with bias) to a bf16 SBUF accumulator in the
                long-idle xvp pool. Doled out as PE filler during pairs 6,7
                (which have no projection work left); the tail then only runs
                i=4..7 + the accumulator add."""

                def __init__(self, j, wosb, woacc):
                    self.ps = ppj.tile([P, D], F32, name="ps_w1", tag="pj")
                    self.j, self.wosb, self.woacc = j, wosb, woacc
                    self.steps = [(c, i) for i in range(4) for c in range(NCH)]
                    self.dst = None

                def step(self):
                    c, i = self.steps.pop(0)
                    nc.tensor.matmul(
                        self.ps[:, ts(c, CH)],
                        aot[i][:, ts(self.j, P)],
                        self.wosb[i][:, ts(c, CH)],
                        start=(i == 0),
                        stop=(i == 3),
                    )
                    if not self.steps:
                        acc = wacc.tile([P, D], BF16, name=f"wa{self.j}", tag="wa")
                        nc.vector.tensor_add(acc[:], self.ps[:], bo_sb[:])
                        self.woacc[self.j] = acc
                    return bool(self.steps)

            def head_qk(h, qt_t, kt_t, jobs, fpp=2):
                """scores -> exp -> mask for head h; returns the 8 E.T tiles.
                After each score tile, emits `fpp` filler matmuls from `jobs`
                (fpp>2 in the early pairs that also carry the V projection)."""
                prow = (h % 2) * 64
                eh = []
                for i in range(NT):
                    st_ps = pst.tile([P, S], F32, name="st", tag="st")
                    for c in range(NCH):
                        nc.tensor.matmul(
                            st_ps[:, ts(c, CH)],
                            kt_t[ds(prow, 64), ts(i, P)],
                            qt_t[ds(prow, 64), ts(c, CH)],
                            start=True,
                            stop=True,
                        )
                    e = epool.tile([P, S], BF16, name=f"e{i}", tag="e")
                    nc.scalar.activation(e[:], st_ps[:], mybir.ActivationFunctionType.Exp)
                    # masks all on GPSIMD: keeps the DVE queue free of ACT-paced
                    # work so the q/k projection evictions (which gate the next
                    # pair's QK) run as soon as their psum completes
                    if mask_on_gpsimd:
                        nc.gpsimd.tensor_mul(e[:], e[:], msk[i][:])
                    else:
                        nc.vector.tensor_mul(e[:], e[:], msk[i][:])
                    eh.append(e)
                    for _ in range(fpp):
                        if jobs and not jobs[0].step():
                            jobs.pop(0)
                return eh

            def head_av(h, eh, aopair):
                prow = (h % 2) * 64
                for j in range(NT):
                    av = ps_av.tile([P, P], F32, name="av", tag="av")
                    for i in range(NT):
                        nc.tensor.matmul(
                            av[:, 0:65],
                            eh[i][:, ts(j, P)],
                            vv[i][:, ds(h * 65, 65)],
                            start=(i == 0),
                            stop=(i == NT - 1),
                        )
                    rc = rpool.tile([P, 1], F32, name="rc", tag="rc")
                    nc.vector.reciprocal(rc[:], av[:, ds(64, 1)])
                    nc.vector.tensor_scalar_mul(
                        aopair[j][:, ds(prow, 64)], av[:, 0:64], rc[:]
                    )

            def transpose_pair(t, aopair):
                # all 8 [128,128]bf16 transposes fit ONE psum bank: 1 slot + 1 big
                # DVE copy instead of 8 of each. (DMA-xbar transposes measured
                # WORSE: they contend with the chained input prefetch on the
                # in-order SP queue.)
                ptb = ps_av.tile([P, S], BF16, name="ptb", tag="av")
                for j in range(NT):
                    nc.tensor.matmul(
                        ptb[:, ts(j, P)],
                        aopair[j][:],
                        ident[:],
                        is_transpose=True,
                        start=(j == 0),
                        stop=(j == NT - 1),
                        skip_group_check=True,
                    )
                nc.vector.tensor_copy(aot[t][:], ptb[:])

            class VChunkJob:
                """V-projection of s-block st_, output chunk c (heads c*8..
                c*8+7): 8 matmuls into a 1-bank ps_av tile + bias-scatter
                eviction into vv[st_]. Doled out as filler in pairs 0-2.
                AV of heads 0-7 (pairs 0-3) only needs c=0, so c=0 jobs run
                in pair 0 and c=1 spreads over pairs 1-2 (heads 8+ AV starts
                at pair 4)."""

                def __init__(self, st_, c):
                    self.ps = ps_av.tile([P, CH], F32, name="ps_pv", tag="av")
                    self.st_, self.c = st_, c
                    self.steps = list(range(NT))

                def step(self):
                    i = self.steps.pop(0)
                    nc.tensor.matmul(
                        self.ps[:],
                        xvsb[i][:, ts(self.st_, P)],
                        wvsb[i][:, ts(self.c, CH)],
                        start=(i == 0),
                        stop=(i == NT - 1),
                    )
                    if not self.steps:
                        g0c = self.c * 8
                        nc.vector.tensor_add(
                            vv[self.st_][:, ds(g0c * 65, 8 * 65)].rearrange(
                                "p (g c) -> p g c", c=65
                            )[:, :, 0:64],
                            self.ps.rearrange("p (g c) -> p g c", c=64),
                            bv_sb[:, ds(g0c * 65, 8 * 65)].rearrange(
                                "p (g c) -> p g c", c=65
                            )[:, :, 0:64],
                        )
                    return bool(self.steps)

            # ---- main loop over head pairs ----
            # static PE order per pair: QK (feeds ACT) -> next-pair projection
            # (fills PE while ACT runs the exps) -> AV(2t) -> previous pair's
            # transposes (extra PE filler before AV(2t+1)'s E is ready) -> AV(2t+1)
            def wo_stile2(j, wosb, woacc):
                # second half (i=4..7) of WO s-tile j + the pass-1 accumulator.
                # psum alternates pst/ppj so eviction of tile j overlaps the
                # matmuls of tile j+1 (scores/projections are done by now).
                pool, tag = (pst, "st") if j % 2 else (ppj, "pj")
                ps = pool.tile([P, D], F32, name="ps_wo", tag=tag)
                for i in range(4, NT):
                    for c in range(NCH):
                        nc.tensor.matmul(
                            ps[:, ts(c, CH)],
                            aot[i][:, ts(j, P)],
                            wosb[i][:, ts(c, CH)],
                            start=(i == 4),
                            stop=(i == NT - 1),
                        )
                # final add + out DMA in 512-chunks: halves the last tile's
                # serial evict->DMA latency (adds on DVE — Pool can't read PSUM)
                osb = opool.tile([P, D], BF16, name="osb", tag="osb")
                for c in range(NCH):
                    nc.vector.tensor_add(
                        osb[:, ts(c, CH)], ps[:, ts(c, CH)], woacc[j][:, ts(c, CH)]
                    )
                    nc.sync.dma_start(out[ts(j, P), ts(c, CH)], osb[:, ts(c, CH)])

            # pair-0 scores start right after q0/k0 project (~+12us instead of
            # ~+45us): the V projection rides as filler in pairs 0-2 and all
            # other projections run ONE pair ahead as filler (just-in-time:
            # pair t+1's q/k evict in the closing periods of pair t).
            qts = {0: project(sl_q, bq_sb, 0, xqsb, "qt")}
            kts = {0: project(sl_k, bk_sb, 0, xksb, "kt")}

            # filler-per-period by pair: pair 0 carries proj(1)+all V c=0
            # (96 steps = 16x6); pairs 1,2 carry proj + half of V c=1 each
            # (64 = 16x4); pair 6 carries proj(7) + WO pass1 j=0..3 (64);
            # pair 7 carries pass1 j=4..7 (32 = 16x2).
            fpp_by_pair = [6, 4, 4, 2, 2, 2, 4, 2]
            prev = None
            wosb = []
            woacc = [None] * NT
            for t in range(NT):
                qt_t, kt_t = qts.pop(t), kts.pop(t)
                aopair = [
                    aop.tile([P, P], BF16, name=f"aop{j}", tag="aop") for j in range(NT)
                ]
                fpp = fpp_by_pair[t]
                jobs = []
                jq = jk = None
                if t + 1 < NT:
                    jq = ProjJob(load_slab(wq, t + 1), bq_sb, t + 1, xqsb, "qt")
                    jobs.append(jq)
                if t == 0:
                    jobs.extend(VChunkJob(st_, 0) for st_ in range(4))
                elif t == 1:
                    jobs.extend(VChunkJob(st_, 1) for st_ in range(2))
                elif t == 2:
                    jobs.extend(VChunkJob(st_, 1) for st_ in range(4, 6))
                elif t == 6:
                    jobs.extend(WoPass1Job(j, wosb, woacc) for j in (0, 1))
                elif t == 7:
                    jobs.extend(WoPass1Job(j, wosb, woacc) for j in (4, 5))
                eh_a = head_qk(2 * t, qt_t, kt_t, jobs, fpp)
                if t + 1 < NT:
                    jk = ProjJob(load_slab(wk, t + 1), bk_sb, t + 1, xksb, "kt")
                    jobs.append(jk)
                if t == 0:
                    jobs.extend(VChunkJob(st_, 0) for st_ in range(4, NT))
                elif t == 1:
                    jobs.extend(VChunkJob(st_, 1) for st_ in range(2, 4))
                elif t == 2:
                    jobs.extend(VChunkJob(st_, 1) for st_ in range(6, NT))
                elif t == 6:
                    jobs.extend(WoPass1Job(j, wosb, woacc) for j in (2, 3))
                elif t == 7:
                    jobs.extend(WoPass1Job(j, wosb, woacc) for j in (6, 7))
                eh_b = head_qk(2 * t + 1, qt_t, kt_t, jobs, fpp)
                while jobs:  # safety drain (steps == slots normally)
                    if not jobs[0].step():
                        jobs.pop(0)
                if t + 1 < NT:
                    qts[t + 1] = jq.dst
                    kts[t + 1] = jk.dst
                if t == 4:
                    # prefetch WO weights
                    for i in range(NT):
                        w_t = wold.tile([P, D], BF16, name=f"wo{i}", tag="wo")
                        nc.sync.dma_start(w_t[:], wo[ts(i, P), :])
                        wosb.append(w_t)
                if prev is not None:
                    transpose_pair(t - 1, prev)
                head_av(2 * t, eh_a, aopair)
                if t < NT - 1:
                    head_av(2 * t + 1, eh_b, aopair)
                else:
                    # tail: interleave the last head's AV with its transposes and
                    # the WO s-tiles so the output projection starts per-j instead
                    # of waiting for the whole pair
                    prow = 64
                    for j in range(NT + 2):
                        if j < NT:
                            av = ps_av.tile([P, P], F32, name="av", tag="av")
                            for i in range(NT):
                                nc.tensor.matmul(
                                    av[:, 0:65],
                                    eh_b[i][:, ts(j, P)],
                                    vv[i][:, ds((2 * t + 1) * 65, 65)],
                                    start=(i == 0),
                                    stop=(i == NT - 1),
                                )
                            rc = rpool.tile([P, 1], F32, name="rc", tag="rc")
                            nc.vector.reciprocal(rc[:], av[:, ds(64, 1)])
                            nc.vector.tensor_scalar_mul(
                                aopair[j][:, ds(prow, 64)], av[:, 0:64], rc[:]
                            )
                        if 1 <= j <= NT:
                            pt = ps_av.tile([P, P], BF16, name="pt", tag="av")
                            nc.tensor.transpose(pt[:], aopair[j - 1][:], ident[:])
                            nc.vector.tensor_copy(aot[t][:, ts(j - 1, P)], pt[:])
                        if j >= 2:
                            wo_stile2(j - 2, wosb, woacc)
                prev = aopair

        for _it in range(iters):
            emit_body()

    nc.compile()
    return nc


def prep_inputs(q, k, v, mask, WQ_w, WQ_b, WK_w, WK_b, WV_w, WV_b, WO_w, WO_b):
    """Build the 8 per-core input maps (host-side layout prep)."""
    f32 = np.float32

    def slabs(wt):  # [D,D] W.T -> [NT, P, D]: [t][p][i*128+f] = wt[i*128+p, t*128+f]
        return np.ascontiguousarray(
            wt.reshape(NT, P, NT, P).transpose(2, 1, 0, 3).reshape(NT, P, D)
        )

    wq_t = slabs((WQ_w.astype(f32) * 0.125).T).astype(NPBF)
    wk_t = slabs(WK_w.astype(f32).T).astype(NPBF)
    wv_t = np.ascontiguousarray(WV_w.astype(f32).T).astype(NPBF)
    wo_t = np.ascontiguousarray(WO_w.astype(f32).T).astype(NPBF)
    bq_l = np.ascontiguousarray((WQ_b.astype(f32) * 0.125).reshape(NT, P).T)
    bk_l = np.ascontiguousarray(WK_b.astype(f32).reshape(NT, P).T)
    bvb = np.zeros((P, H * 65), NPBF)
    bv_f = WV_b.astype(f32)
    for h in range(H):
        bvb[:, h * 65 : h * 65 + 64] = bv_f[h * 64 : (h + 1) * 64].astype(NPBF)[None, :]
    bob = np.ascontiguousarray(np.broadcast_to(WO_b.astype(f32), (P, D))).astype(NPBF)

    in_maps = []
    for b in range(B):
        in_maps.append(
            {
                "xq": np.ascontiguousarray(q[b].astype(f32).T).astype(NPBF),
                "xk": np.ascontiguousarray(k[b].astype(f32).T).astype(NPBF),
                "xv": np.ascontiguousarray(v[b].astype(f32).T).astype(NPBF),
                "wq": wq_t,
                "wk": wk_t,
                "wv": wv_t,
                "wo": wo_t,
                "bq": bq_l,
                "bk": bk_l,
                "bvb": bvb,
                "bob": bob,
                "mt": np.ascontiguousarray(mask[b, 0].T.astype(f32)).astype(NPBF),
            }
        )
    return in_maps


def _ensure_neuron_backend():
    # if jax was already initialized cpu-only (e.g. JAX_PLATFORMS=cpu was set
    # before this module was imported), re-discover the neuron/axon backend
    import jax

    try:
        if all(d.platform == "cpu" for d in jax.devices()):
            jax.clear_backends()
    except Exception:
        pass


def kernel(q, k, v, mask, WQ_w, WQ_b, WK_w, WK_b, WV_w, WV_b, WO_w, WO_b):
    global _NC_CACHE, LAST_RESULTS
    _ensure_neuron_backend()
    if _NC_CACHE is None:
        _NC_CACHE = build_nc()
    nc = _NC_CACHE
    in_maps = prep_inputs(
        q, k, v, mask, WQ_w, WQ_b, WK_w, WK_b, WV_w, WV_b, WO_w, WO_b
    )
    res = run_bass_kernel_spmd(nc, in_maps, core_ids=list(range(B)))
    LAST_RESULTS = res
    return np.stack([res.results[b]["out"] for b in range(B)], axis=0).astype(
        np.float32
    )

